# revision 17
# baseline (speedup 1.0000x reference)
"""KANLinear (RBF-KAN) Trainium2 kernel — fp8 DoubleRow version.

Math (matches the reference):
  x_flat [B=8192, IN=1024]
  base   = silu(x) @ base_w.T + base_b
  basis[b,i,g] = exp(-(d*(x[b,i]-grid[g]))**2),  grid = linspace(-2,2,8)
  spline = einsum('big,oig->bo', basis, spline_w)
  out    = base + spline

Implementation (data-parallel over tokens, 8 cores x 1024 tokens):
  - The spline contraction runs in fp8 e4m3 with perf_mode=DoubleRow:
    each MM contracts a PAIR of 128-row k-subtiles in the time a bf16
    MM contracts one (the PE moving port is byte-bound at 2B/cycle, and
    fp8 carries 2x K per byte). Spline MMs: 32 pairs x 8 m x 2 n = 512.
  - Accuracy (plain e4m3 is ~2.7% rel err, gate 2e-2):
      1. Variance reduction: quantize the RESIDUAL basis
         resid = basis - c_g - beta_g*silu(x); (c_g, beta_g) are
         least-squares fits (hardcoded). c folds into a host bias,
         beta folds into the base weights. |resid| ~ 0.68 |basis|.
      2. W-side: error-feedback (GPTQ-style) e4m3 rounding of W across
         the 8 correlated g-rows (host-side, uses the shared 8x8 resid
         Gram), plus optional explicit W_lo correction matmuls for the
         highest-variance grid points (LO_COVER knob).
  - Per core: 4 groups x 256 tokens; PSUM holds 2 groups -> seamless
    group overlap. basis via one DVE stt (v=(x-2g)*x) + one ACT Exp;
    silu2 = x*(1+tanh(x/2)); everything stays in the exp/tanh table.
  - base_b and the c-fold bias are added on the host; outputs are
    accumulated at WSCALE and divided on the host.
"""

import os
import sys

os.environ.setdefault("MYCRO_LOCAL_CACHE", "1")
for _p in ("/opt/trn_rl_repo", "/root/.axon_site/_ro/trn_rl_repo"):
    if os.path.isdir(_p) and _p not in sys.path:
        sys.path.insert(0, _p)

import numpy as np
import ml_dtypes

F8NP = ml_dtypes.float8_e4m3
BFNP = ml_dtypes.bfloat16

IN_F = 1024
OUT_F = 1024
G = 8
GRID_LO, GRID_HI = -2.0, 2.0
NCORES = 8
TOK = 8192
TCORE = TOK // NCORES     # 1024 tokens per core
NG = 4                    # token groups per core
GTOK = TCORE // NG        # 256 tokens per group
MT = GTOK // 128          # 2 psum m-tiles per group
KB = IN_F // 128          # 8 i-blocks
KS = G * KB               # 64 k-subtiles of 128
NPAIR = KS // 2           # 32 DoubleRow k-pairs
UPG = KB // 2             # 4 pairs per g-block

WSCALE = 64.0

_DELTA = float((GRID_HI - GRID_LO) / (G - 1))
_D = 1.0 / (_DELTA + 1e-6)
_D2 = _D * _D
_GRID = np.linspace(GRID_LO, GRID_HI, G, dtype=np.float32).astype(np.float64)

# Least-squares fit of basis_g(x) ~ c_g + beta_g*silu(x) over x ~ N(0,1)
# (bf16-rounded silu). Computed offline on the reference distribution.
_C = np.array([0.08754251, 0.20408037, 0.3485522, 0.42897628,
               0.37042523, 0.21574167, 0.07760693, 0.01135657])
_BETA = np.array([-0.09874898, -0.23330925, -0.36547238, -0.32523782,
                  -0.04140068, 0.27750214, 0.37861404, 0.270346])

CENTER = True
GPTQ_W = False  # measured: no gain (resid columns ~uncorrelated across g)
LO_COVER = (3, 4)         # grid points with explicit W_lo correction MMs

TRACE = False
LAST_RESULT = None
_NC_CACHE = None


def build_nc():
    from concourse import bacc
    import concourse.mybir as mybir
    import concourse.tile as tile

    F32 = mybir.dt.float32
    BF16 = mybir.dt.bfloat16
    F8 = mybir.dt.float8e4
    Alu = mybir.AluOpType
    Act = mybir.ActivationFunctionType
    DR = mybir.MatmulPerfMode.DoubleRow

    ncov = len(LO_COVER)
    nc = bacc.Bacc("TRN2", target_bir_lowering=False)
    xg_d = nc.dram_tensor("xg", [NG, 128, KB, GTOK], BF16, kind="ExternalInput")
    whi_d = nc.dram_tensor("whi", [128, NPAIR, 2, OUT_F], F8, kind="ExternalInput")
    bw_d = nc.dram_tensor("basew", [128, KB, OUT_F], BF16, kind="ExternalInput")
    out_d = nc.dram_tensor("out", [TCORE, OUT_F], F32, kind="ExternalOutput")
    if ncov:
        wlo_d = nc.dram_tensor("wlo", [128, ncov * UPG, 2, OUT_F], F8,
                               kind="ExternalInput")

    def exp_bias(g):
        gval = float(_GRID[g])
        return float(-_D2 * gval * gval)

    # activation() requires pre-registered [128,1] const APs for fp biases
    def register_const_ap(value):
        t = nc.alloc_sbuf_tensor(f"const-bias-{value}", [128, 1], F32)
        nc.gpsimd.memset(t.ap(), value)
        nc.const_aps.aps[(F32, value)] = t.ap()

    need = set()
    for g in range(G):
        need.add(float(-_D * _GRID[g]))   # Square bias (path A)
        need.add(exp_bias(g))             # Exp bias (path B)
    for value in sorted(need):
        register_const_ap(value)
    nc.all_engine_barrier()

    with tile.TileContext(nc) as tc:
        with (
            tc.tile_pool(name="const", bufs=1) as cpool,
            tc.tile_pool(name="xg", bufs=3) as xpool,
            tc.tile_pool(name="silu", bufs=2) as spool,
            tc.tile_pool(name="vsc", bufs=2) as vpool,
            tc.tile_pool(name="bsc", bufs=2) as bpool,
            tc.tile_pool(name="r1s", bufs=2) as rpool,
            tc.tile_pool(name="resid", bufs=8) as fpool,
            tc.tile_pool(name="osb", bufs=2) as opool,
            tc.tile_pool(name="psum", bufs=4, space="PSUM") as ppool,
        ):
            whi_sb = cpool.tile([128, NPAIR, 2, OUT_F], F8)
            if ncov:
                wlo_sb = cpool.tile([128, ncov * UPG, 2, OUT_F], F8)
            bw_sb = cpool.tile([128, KB, OUT_F], BF16)
            ones_sb = cpool.tile([1, 128], BF16)

            pending = []

            def emit_evictions():
                for ps_t, row in pending:
                    o = opool.tile([128, OUT_F], F32, tag="osb", name=f"o_{row}")
                    nc.vector.tensor_copy(o[:, 0:512], ps_t[:, 0:512])
                    nc.scalar.copy(o[:, 512:1024], ps_t[:, 512:1024])
                    nc.sync.dma_start(out_d[row * 128:(row + 1) * 128, :], o[:])
                pending.clear()

            xg_t = {}
            silu2_t = {}
            resid_t = {}
            ps_t = {}

            def emit_xg_dma(grp, halves=1):
                xg = xpool.tile([128, KB, GTOK], BF16, tag="xg", name=f"xg{grp}")
                xg_t[grp] = xg
                if halves == 1:
                    nc.sync.dma_start(xg[:], xg_d[grp])
                else:
                    h = KB // halves
                    for i in range(halves):
                        nc.sync.dma_start(xg[:, i * h:(i + 1) * h, :],
                                          xg_d[grp, :, i * h:(i + 1) * h, :])

            def emit_silu(grp, halves=1):
                xg = xg_t[grp]
                silu2 = spool.tile([128, KB, GTOK], BF16, tag="silu",
                                   name=f"s2{grp}")
                silu2_t[grp] = silu2
                h = KB // halves
                for i in range(halves):
                    xpart = xg[:, i * h:(i + 1) * h, :].rearrange(
                        "p k t -> p (k t)")
                    th = vpool.tile([128, h * GTOK], F32, tag="vsc",
                                    name=f"th{grp}_{i}")
                    nc.scalar.activation(th[:], xpart, Act.Tanh, scale=0.5)
                    nc.vector.scalar_tensor_tensor(
                        silu2[:, i * h:(i + 1) * h, :].rearrange(
                            "p k t -> p (k t)"),
                        th[:], 1.0, xpart, op0=Alu.add, op1=Alu.mult,
                    )

            def emit_chain(grp, g):
                # basis argument: path A squares on ACT, path B on DVE,
                # to balance engine load (Pool lacks stt/tt on TRN2).
                xflat = xg_t[grp][:].rearrange("p k t -> p (k t)")
                v = vpool.tile([128, KB * GTOK], F32, tag="vsc",
                               name=f"v{grp}_{g}")
                path_a = True  # all squares on ACT: DVE is the scarcer engine
                if path_a:
                    nc.scalar.activation(
                        v[:], xflat, Act.Square,
                        bias=float(-_D * _GRID[g]), scale=float(_D),
                    )
                else:
                    nc.vector.scalar_tensor_tensor(
                        v[:], xflat, -2.0 * float(_GRID[g]), xflat,
                        op0=Alu.add, op1=Alu.mult,
                    )
                resid = fpool.tile([128, KB, GTOK], F8, tag="resid",
                                   name=f"r{grp}_{g}")
                resid_t[(grp, g)] = resid
                rflat = resid[:].rearrange("p k t -> p (k t)")
                escale = -1.0 if path_a else float(-_D2)
                ebias = 0.0 if path_a else exp_bias(g)
                if CENTER:
                    basis = bpool.tile([128, KB * GTOK], F32, tag="bsc",
                                       name=f"b{grp}_{g}")
                    nc.scalar.activation(
                        basis[:], v[:], Act.Exp, bias=ebias, scale=escale,
                    )
                    # bs = (beta/2)*silu2 + c   (Pool ts, 1-input)
                    bs = rpool.tile([128, KB * GTOK], F32, tag="r1s",
                                    name=f"bs_{grp}_{g}")
                    nc.gpsimd.tensor_scalar(
                        bs[:], silu2_t[grp][:].rearrange("p k t -> p (k t)"),
                        float(_BETA[g] / 2.0), float(_C[g]),
                        op0=Alu.mult, op1=Alu.add,
                    )
                    # resid8 = basis - bs, cast e4m3 (DVE tt)
                    nc.vector.tensor_tensor(
                        rflat, basis[:], bs[:], op=Alu.subtract,
                    )
                else:
                    nc.scalar.activation(
                        rflat, v[:], Act.Exp, bias=ebias, scale=escale,
                    )

            def emit_base(grp):
                ps = [
                    ppool.tile([128, OUT_F], F32, tag="ps",
                               name=f"ps_g{grp}m{m}")
                    for m in range(MT)
                ]
                ps_t[grp] = ps
                silu2 = silu2_t[grp]
                for m in range(MT):
                    for kb in range(KB):
                        lhsT = silu2[:, kb, m * 128:(m + 1) * 128]
                        for n in range(2):
                            nc.tensor.matmul(
                                ps[m][:, n * 512:(n + 1) * 512],
                                lhsT,
                                bw_sb[:, kb, n * 512:(n + 1) * 512],
                                start=(kb == 0), stop=False,
                            )

            def emit_spline(grp, g):
                ps = ps_t[grp]
                resid = resid_t.pop((grp, g))
                covered = g in LO_COVER
                cov_idx = sum(1 for c in LO_COVER if c < g)
                last_g = g == G - 1
                for u in range(UPG):
                    t = g * UPG + u
                    for m in range(MT):
                        lhsT = resid[:, 2 * u:2 * u + 2, m * 128:(m + 1) * 128]
                        for n in range(2):
                            last_mm = last_g and u == UPG - 1 and not covered
                            nc.tensor.matmul(
                                ps[m][:, n * 512:(n + 1) * 512],
                                lhsT,
                                whi_sb[:, t, :, n * 512:(n + 1) * 512],
                                start=False,
                                stop=last_mm,
                                perf_mode=DR,
                            )
                            if covered:
                                nc.tensor.matmul(
                                    ps[m][:, n * 512:(n + 1) * 512],
                                    lhsT,
                                    wlo_sb[:, cov_idx * UPG + u, :,
                                           n * 512:(n + 1) * 512],
                                    start=False,
                                    stop=(last_g and u == UPG - 1),
                                    perf_mode=DR,
                                )
                if last_g:
                    for m in range(MT):
                        pending.append((ps[m], grp * MT + m))
                if g == 1 and pending:
                    emit_evictions()

            # ---- chain-unit and mm-unit streams, chain runs LOOKAHEAD ahead
            # silu(grp+1) is hoisted to mid-grp so the next group's base
            # matmuls never wait on the ACT backlog.
            chain_units = [lambda: emit_silu(0, halves=2)]
            for grp in range(NG):
                for g in range(G):
                    if g == 0 and grp + 1 < NG:
                        chain_units.append(lambda grp=grp: emit_xg_dma(grp + 1))
                    chain_units.append(lambda grp=grp, g=g: emit_chain(grp, g))
                    if g == 3 and grp + 1 < NG:
                        chain_units.append(lambda grp=grp: emit_silu(grp + 1))

            mm_units = []
            for grp in range(NG):
                mm_units.append(lambda grp=grp: emit_base(grp))
                for g in range(G):
                    mm_units.append(lambda grp=grp, g=g: emit_spline(grp, g))

            # prologue: warmup junk MMs (HAM) + initial DMAs
            nc.vector.memset(ones_sb[:], 1.0)
            warm_ps = ppool.tile([128, OUT_F], F32, tag="ps", name="warm")
            for w in range(20):
                nc.tensor.matmul(
                    warm_ps[:, 0:128], ones_sb[0:1, :], ones_sb[0:1, :],
                    start=True, stop=True,
                )
            emit_xg_dma(0, halves=2)
            nc.sync.dma_start(bw_sb[:], bw_d[:])
            for t in range(4):
                nc.sync.dma_start(whi_sb[:, t], whi_d[:, t])
            if ncov:
                nc.sync.dma_start(wlo_sb[:], wlo_d[:])
            for t in range(4, NPAIR):
                nc.sync.dma_start(whi_sb[:, t], whi_d[:, t])

            LOOKAHEAD = 4
            ci = 0
            for j, mm in enumerate(mm_units):
                target = min(len(chain_units),
                             (j * len(chain_units)) // len(mm_units) + LOOKAHEAD)
                while ci < target:
                    chain_units[ci]()
                    ci += 1
                mm()
            while ci < len(chain_units):
                chain_units[ci]()
                ci += 1
            emit_evictions()

    nc.compile()
    return nc


def _quantize_w(spline_w):
    """e4m3 quantization of W*WSCALE with optional GPTQ-style error
    feedback across the g dimension (shared Gram, hardcoded)."""
    W = spline_w.astype(np.float64) * WSCALE  # [O, I, G]
    if not GPTQ_W:
        Whi = (W.astype(np.float32)).astype(F8NP)
        return Whi
    # Shared 8x8 Gram of the centered residual basis under x~N(0,1).
    # E[resid_g resid_g'] computed offline on the reference distribution.
    H = _RESID_GRAM + 1e-6 * np.trace(_RESID_GRAM) / G * np.eye(G)
    Hinv = np.linalg.inv(H)
    U = np.linalg.cholesky(Hinv[::-1, ::-1].T)[::-1, ::-1].T
    Wp = W.copy()
    Q = np.zeros(W.shape, dtype=F8NP)
    for k in range(G):
        Q[:, :, k] = Wp[:, :, k].astype(np.float32).astype(F8NP)
        err = Wp[:, :, k] - Q[:, :, k].astype(np.float64)
        if k + 1 < G:
            coef = U[k, k + 1:] / U[k, k]
            Wp[:, :, k + 1:] -= err[:, :, None] * coef[None, None, :]
    return Q


# E[resid_g resid_g'] for resid = basis - c - beta*silu, x ~ N(0,1).
# (filled in below by calibration; placeholder identity keeps GPTQ sane
# if calibration is skipped)
_RESID_GRAM = np.eye(G) * 0.05


def _host_prep(x, base_w, base_b, spline_w):
    x = np.asarray(x, dtype=np.float32)
    base_w = np.asarray(base_w, dtype=np.float32)
    base_b = np.asarray(base_b, dtype=np.float32)
    spline_w = np.asarray(spline_w, dtype=np.float32)

    x_flat = np.ascontiguousarray(x.reshape(TOK, IN_F)).astype(BFNP)

    # spline weights: k = g*IN + i  ->  [G*IN, OUT]
    W8 = _quantize_w(spline_w)  # [O, I, G] e4m3 (scaled by WSCALE)
    Wk_hi = W8.transpose(2, 1, 0).reshape(G * IN_F, OUT_F)

    def pack(Wm):  # [G*IN, OUT] -> [128, NPAIR, 2, OUT]
        return np.ascontiguousarray(
            Wm.reshape(NPAIR, 2, 128, OUT_F).transpose(2, 0, 1, 3))

    whi = pack(Wk_hi)

    ncov = len(LO_COVER)
    wlo = None
    if ncov:
        Wk = spline_w.transpose(2, 1, 0).reshape(G * IN_F, OUT_F).astype(np.float64)
        Wlo_full = ((Wk * WSCALE - Wk_hi.astype(np.float64))
                    .astype(np.float32).astype(F8NP))
        parts = []
        for g in sorted(LO_COVER):
            blk = Wlo_full[g * IN_F:(g + 1) * IN_F]  # [IN, OUT]
            parts.append(blk.reshape(UPG, 2, 128, OUT_F).transpose(2, 0, 1, 3))
        wlo = np.ascontiguousarray(np.concatenate(parts, axis=1))

    # base weights with beta-fold, 0.5 silu2 factor, and WSCALE
    if CENTER:
        V = np.einsum("g,oig->io", _BETA, spline_w.astype(np.float64))
    else:
        V = 0.0
    bw_eff = 0.5 * (base_w.T.astype(np.float64) + V) * WSCALE
    bw = np.ascontiguousarray(
        bw_eff.reshape(KB, 128, OUT_F).transpose(1, 0, 2)).astype(BFNP)

    if CENTER:
        bias = (_C[None, :] * spline_w.astype(np.float64).sum(axis=1)).sum(axis=1)
    else:
        bias = np.zeros(OUT_F)
    bias = bias + base_b.astype(np.float64)

    in_maps = []
    for c in range(NCORES):
        shard = x_flat[c * TCORE:(c + 1) * TCORE, :]
        xT = shard.T  # [in, tok]
        xg = np.ascontiguousarray(
            xT.reshape(KB, 128, NG, GTOK).transpose(2, 1, 0, 3))
        m = {"xg": xg, "whi": whi, "basew": bw}
        if ncov:
            m["wlo"] = wlo
        in_maps.append(m)
    return in_maps, bias


def kernel(x, base_w, base_b, spline_w):
    global _NC_CACHE, LAST_RESULT
    from concourse.bass_utils import run_bass_kernel_spmd

    in_maps, bias = _host_prep(x, base_w, base_b, spline_w)
    if _NC_CACHE is None:
        _NC_CACHE = build_nc()
    res = run_bass_kernel_spmd(
        _NC_CACHE, in_maps, core_ids=list(range(NCORES)), trace=TRACE
    )
    LAST_RESULT = res
    outs = [np.asarray(r["out"], dtype=np.float64) for r in res.results]
    full = np.concatenate(outs, axis=0) / WSCALE + bias
    return full.astype(np.float32).reshape(4, 2048, OUT_F)


# revision 18
# speedup vs baseline: 1.0623x; 1.0623x over previous
"""KANLinear (RBF-KAN) Trainium2 kernel — fp8 DoubleRow version.

Math (matches the reference):
  x_flat [B=8192, IN=1024]
  base   = silu(x) @ base_w.T + base_b
  basis[b,i,g] = exp(-(d*(x[b,i]-grid[g]))**2),  grid = linspace(-2,2,8)
  spline = einsum('big,oig->bo', basis, spline_w)
  out    = base + spline

Implementation (data-parallel over tokens, 8 cores x 1024 tokens):
  - The spline contraction runs in fp8 e4m3 with perf_mode=DoubleRow:
    each MM contracts a PAIR of 128-row k-subtiles in the time a bf16
    MM contracts one (the PE moving port is byte-bound at 2B/cycle, and
    fp8 carries 2x K per byte). Spline MMs: 32 pairs x 8 m x 2 n = 512.
  - Accuracy (plain e4m3 is ~2.7% rel err, gate 2e-2):
      1. Variance reduction: quantize the RESIDUAL basis
         resid = basis - c_g - beta_g*silu(x); (c_g, beta_g) are
         least-squares fits (hardcoded). c folds into a host bias,
         beta folds into the base weights. |resid| ~ 0.68 |basis|.
      2. W-side: error-feedback (GPTQ-style) e4m3 rounding of W across
         the 8 correlated g-rows (host-side, uses the shared 8x8 resid
         Gram), plus optional explicit W_lo correction matmuls for the
         highest-variance grid points (LO_COVER knob).
  - Per core: 4 groups x 256 tokens; PSUM holds 2 groups -> seamless
    group overlap. basis via one DVE stt (v=(x-2g)*x) + one ACT Exp;
    silu2 = x*(1+tanh(x/2)); everything stays in the exp/tanh table.
  - base_b and the c-fold bias are added on the host; outputs are
    accumulated at WSCALE and divided on the host.
"""

import os
import sys

os.environ.setdefault("MYCRO_LOCAL_CACHE", "1")
for _p in ("/opt/trn_rl_repo", "/root/.axon_site/_ro/trn_rl_repo"):
    if os.path.isdir(_p) and _p not in sys.path:
        sys.path.insert(0, _p)

import numpy as np
import ml_dtypes

F8NP = ml_dtypes.float8_e4m3
BFNP = ml_dtypes.bfloat16

IN_F = 1024
OUT_F = 1024
G = 8
GRID_LO, GRID_HI = -2.0, 2.0
NCORES = 8
TOK = 8192
TCORE = TOK // NCORES     # 1024 tokens per core
NG = 4                    # token groups per core
GTOK = TCORE // NG        # 256 tokens per group
MT = GTOK // 128          # 2 psum m-tiles per group
KB = IN_F // 128          # 8 i-blocks
KS = G * KB               # 64 k-subtiles of 128
NPAIR = KS // 2           # 32 DoubleRow k-pairs
UPG = KB // 2             # 4 pairs per g-block

WSCALE = 64.0

_DELTA = float((GRID_HI - GRID_LO) / (G - 1))
_D = 1.0 / (_DELTA + 1e-6)
_D2 = _D * _D
_GRID = np.linspace(GRID_LO, GRID_HI, G, dtype=np.float32).astype(np.float64)

# Least-squares fit of basis_g(x) ~ c_g + beta_g*silu(x) over x ~ N(0,1)
# (bf16-rounded silu). Computed offline on the reference distribution.
_C = np.array([0.08754251, 0.20408037, 0.3485522, 0.42897628,
               0.37042523, 0.21574167, 0.07760693, 0.01135657])
_BETA = np.array([-0.09874898, -0.23330925, -0.36547238, -0.32523782,
                  -0.04140068, 0.27750214, 0.37861404, 0.270346])

CENTER = True
GPTQ_W = False  # measured: no gain (resid columns ~uncorrelated across g)
LO_COVER = (3, 4)         # grid points with explicit W_lo correction MMs

TRACE = False
LAST_RESULT = None
_NC_CACHE = None


def build_nc():
    from concourse import bacc
    import concourse.mybir as mybir
    import concourse.tile as tile

    F32 = mybir.dt.float32
    BF16 = mybir.dt.bfloat16
    F8 = mybir.dt.float8e4
    Alu = mybir.AluOpType
    Act = mybir.ActivationFunctionType
    DR = mybir.MatmulPerfMode.DoubleRow

    ncov = len(LO_COVER)
    nc = bacc.Bacc("TRN2", target_bir_lowering=False)
    xg_d = nc.dram_tensor("xg", [NG, 128, KB, GTOK], BF16, kind="ExternalInput")
    whi_d = nc.dram_tensor("whi", [128, NPAIR, 2, OUT_F], F8, kind="ExternalInput")
    bw_d = nc.dram_tensor("basew", [128, KB, OUT_F], BF16, kind="ExternalInput")
    out_d = nc.dram_tensor("out", [TCORE, OUT_F], F32, kind="ExternalOutput")
    if ncov:
        wlo_d = nc.dram_tensor("wlo", [128, ncov * UPG, 2, OUT_F], F8,
                               kind="ExternalInput")

    def exp_bias(g):
        gval = float(_GRID[g])
        return float(-_D2 * gval * gval)

    # activation() requires pre-registered [128,1] const APs for fp biases
    def register_const_ap(value):
        t = nc.alloc_sbuf_tensor(f"const-bias-{value}", [128, 1], F32)
        nc.gpsimd.memset(t.ap(), value)
        nc.const_aps.aps[(F32, value)] = t.ap()

    need = set()
    for g in range(G):
        need.add(float(-_D * _GRID[g]))   # Square bias (path A)
        need.add(exp_bias(g))             # Exp bias (path B)
    for value in sorted(need):
        register_const_ap(value)
    nc.all_engine_barrier()

    with tile.TileContext(nc) as tc:
        with (
            tc.tile_pool(name="const", bufs=1) as cpool,
            tc.tile_pool(name="xg", bufs=3) as xpool,
            tc.tile_pool(name="silu", bufs=2) as spool,
            tc.tile_pool(name="vsc", bufs=2) as vpool,
            tc.tile_pool(name="bsc", bufs=2) as bpool,
            tc.tile_pool(name="r1s", bufs=2) as rpool,
            tc.tile_pool(name="resid", bufs=8) as fpool,
            tc.tile_pool(name="osb", bufs=2) as opool,
            tc.tile_pool(name="psum", bufs=4, space="PSUM") as ppool,
        ):
            whi_sb = cpool.tile([128, NPAIR, 2, OUT_F], F8)
            if ncov:
                wlo_sb = cpool.tile([128, ncov * UPG, 2, OUT_F], F8)
            bw_sb = cpool.tile([128, KB, OUT_F], BF16)
            ones_sb = cpool.tile([1, 128], BF16)

            pending = []

            def emit_evictions():
                for ps_t, row in pending:
                    o = opool.tile([128, OUT_F], F32, tag="osb", name=f"o_{row}")
                    nc.vector.tensor_copy(o[:, 0:512], ps_t[:, 0:512])
                    nc.scalar.copy(o[:, 512:1024], ps_t[:, 512:1024])
                    nc.sync.dma_start(out_d[row * 128:(row + 1) * 128, :], o[:])
                pending.clear()

            xg_t = {}
            silu2_t = {}
            resid_t = {}
            ps_t = {}

            def emit_xg_dma(grp, halves=1):
                xg = xpool.tile([128, KB, GTOK], BF16, tag="xg", name=f"xg{grp}")
                xg_t[grp] = xg
                if halves == 1:
                    nc.sync.dma_start(xg[:], xg_d[grp])
                else:
                    h = KB // halves
                    for i in range(halves):
                        nc.sync.dma_start(xg[:, i * h:(i + 1) * h, :],
                                          xg_d[grp, :, i * h:(i + 1) * h, :])

            def emit_silu(grp, halves=1):
                xg = xg_t[grp]
                silu2 = spool.tile([128, KB, GTOK], BF16, tag="silu",
                                   name=f"s2{grp}")
                silu2_t[grp] = silu2
                h = KB // halves
                for i in range(halves):
                    xpart = xg[:, i * h:(i + 1) * h, :].rearrange(
                        "p k t -> p (k t)")
                    th = vpool.tile([128, h * GTOK], F32, tag="vsc",
                                    name=f"th{grp}_{i}")
                    nc.scalar.activation(th[:], xpart, Act.Tanh, scale=0.5)
                    nc.vector.scalar_tensor_tensor(
                        silu2[:, i * h:(i + 1) * h, :].rearrange(
                            "p k t -> p (k t)"),
                        th[:], 1.0, xpart, op0=Alu.add, op1=Alu.mult,
                    )

            def emit_chain(grp, g):
                # basis argument: path A squares on ACT, path B on DVE,
                # to balance engine load (Pool lacks stt/tt on TRN2).
                xflat = xg_t[grp][:].rearrange("p k t -> p (k t)")
                v = vpool.tile([128, KB * GTOK], F32, tag="vsc",
                               name=f"v{grp}_{g}")
                path_a = g % 3 != 0  # squares: 5/8 on ACT, 3/8 on DVE
                if path_a:
                    nc.scalar.activation(
                        v[:], xflat, Act.Square,
                        bias=float(-_D * _GRID[g]), scale=float(_D),
                    )
                else:
                    nc.vector.scalar_tensor_tensor(
                        v[:], xflat, -2.0 * float(_GRID[g]), xflat,
                        op0=Alu.add, op1=Alu.mult,
                    )
                resid = fpool.tile([128, KB, GTOK], F8, tag="resid",
                                   name=f"r{grp}_{g}")
                resid_t[(grp, g)] = resid
                rflat = resid[:].rearrange("p k t -> p (k t)")
                escale = -1.0 if path_a else float(-_D2)
                ebias = 0.0 if path_a else exp_bias(g)
                if CENTER:
                    basis = bpool.tile([128, KB * GTOK], F32, tag="bsc",
                                       name=f"b{grp}_{g}")
                    nc.scalar.activation(
                        basis[:], v[:], Act.Exp, bias=ebias, scale=escale,
                    )
                    # bs = (beta/2)*silu2 + c   (Pool ts, 1-input)
                    bs = rpool.tile([128, KB * GTOK], F32, tag="r1s",
                                    name=f"bs_{grp}_{g}")
                    nc.gpsimd.tensor_scalar(
                        bs[:], silu2_t[grp][:].rearrange("p k t -> p (k t)"),
                        float(_BETA[g] / 2.0), float(_C[g]),
                        op0=Alu.mult, op1=Alu.add,
                    )
                    # resid8 = basis - bs, cast e4m3 (DVE tt)
                    nc.vector.tensor_tensor(
                        rflat, basis[:], bs[:], op=Alu.subtract,
                    )
                else:
                    nc.scalar.activation(
                        rflat, v[:], Act.Exp, bias=ebias, scale=escale,
                    )

            def emit_base(grp):
                ps = [
                    ppool.tile([128, OUT_F], F32, tag="ps",
                               name=f"ps_g{grp}m{m}")
                    for m in range(MT)
                ]
                ps_t[grp] = ps
                silu2 = silu2_t[grp]
                for m in range(MT):
                    for kb in range(KB):
                        lhsT = silu2[:, kb, m * 128:(m + 1) * 128]
                        for n in range(2):
                            nc.tensor.matmul(
                                ps[m][:, n * 512:(n + 1) * 512],
                                lhsT,
                                bw_sb[:, kb, n * 512:(n + 1) * 512],
                                start=(kb == 0), stop=False,
                            )

            def emit_spline(grp, g):
                ps = ps_t[grp]
                resid = resid_t.pop((grp, g))
                covered = g in LO_COVER
                cov_idx = sum(1 for c in LO_COVER if c < g)
                last_g = g == G - 1
                for u in range(UPG):
                    t = g * UPG + u
                    for m in range(MT):
                        lhsT = resid[:, 2 * u:2 * u + 2, m * 128:(m + 1) * 128]
                        for n in range(2):
                            last_mm = last_g and u == UPG - 1 and not covered
                            nc.tensor.matmul(
                                ps[m][:, n * 512:(n + 1) * 512],
                                lhsT,
                                whi_sb[:, t, :, n * 512:(n + 1) * 512],
                                start=False,
                                stop=last_mm,
                                perf_mode=DR,
                            )
                            if covered:
                                nc.tensor.matmul(
                                    ps[m][:, n * 512:(n + 1) * 512],
                                    lhsT,
                                    wlo_sb[:, cov_idx * UPG + u, :,
                                           n * 512:(n + 1) * 512],
                                    start=False,
                                    stop=(last_g and u == UPG - 1),
                                    perf_mode=DR,
                                )
                if last_g:
                    for m in range(MT):
                        pending.append((ps[m], grp * MT + m))
                if g == 1 and pending:
                    emit_evictions()

            # ---- chain-unit and mm-unit streams, chain runs LOOKAHEAD ahead
            # silu(grp+1) is hoisted to mid-grp so the next group's base
            # matmuls never wait on the ACT backlog.
            chain_units = []
            for grp in range(NG):
                chain_units.append(lambda grp=grp: emit_silu(grp))
                for g in range(G):
                    if g == 0 and grp + 1 < NG:
                        chain_units.append(lambda grp=grp: emit_xg_dma(grp + 1))
                    chain_units.append(lambda grp=grp, g=g: emit_chain(grp, g))

            mm_units = []
            for grp in range(NG):
                mm_units.append(lambda grp=grp: emit_base(grp))
                for g in range(G):
                    mm_units.append(lambda grp=grp, g=g: emit_spline(grp, g))

            # prologue: warmup junk MMs (HAM) + initial DMAs
            nc.vector.memset(ones_sb[:], 1.0)
            warm_ps = ppool.tile([128, OUT_F], F32, tag="ps", name="warm")
            for w in range(32):
                nc.tensor.matmul(
                    warm_ps[:, 0:128], ones_sb[0:1, :], ones_sb[0:1, :],
                    start=True, stop=True,
                )
            emit_xg_dma(0)
            nc.sync.dma_start(bw_sb[:], bw_d[:])
            for t in range(4):
                nc.sync.dma_start(whi_sb[:, t], whi_d[:, t])
            if ncov:
                nc.sync.dma_start(wlo_sb[:], wlo_d[:])
            for t in range(4, NPAIR):
                nc.sync.dma_start(whi_sb[:, t], whi_d[:, t])

            LOOKAHEAD = 4
            ci = 0
            for j, mm in enumerate(mm_units):
                target = min(len(chain_units),
                             (j * len(chain_units)) // len(mm_units) + LOOKAHEAD)
                while ci < target:
                    chain_units[ci]()
                    ci += 1
                mm()
            while ci < len(chain_units):
                chain_units[ci]()
                ci += 1
            emit_evictions()

    nc.compile()
    return nc


def _quantize_w(spline_w):
    """e4m3 quantization of W*WSCALE with optional GPTQ-style error
    feedback across the g dimension (shared Gram, hardcoded)."""
    W = spline_w.astype(np.float64) * WSCALE  # [O, I, G]
    if not GPTQ_W:
        Whi = (W.astype(np.float32)).astype(F8NP)
        return Whi
    # Shared 8x8 Gram of the centered residual basis under x~N(0,1).
    # E[resid_g resid_g'] computed offline on the reference distribution.
    H = _RESID_GRAM + 1e-6 * np.trace(_RESID_GRAM) / G * np.eye(G)
    Hinv = np.linalg.inv(H)
    U = np.linalg.cholesky(Hinv[::-1, ::-1].T)[::-1, ::-1].T
    Wp = W.copy()
    Q = np.zeros(W.shape, dtype=F8NP)
    for k in range(G):
        Q[:, :, k] = Wp[:, :, k].astype(np.float32).astype(F8NP)
        err = Wp[:, :, k] - Q[:, :, k].astype(np.float64)
        if k + 1 < G:
            coef = U[k, k + 1:] / U[k, k]
            Wp[:, :, k + 1:] -= err[:, :, None] * coef[None, None, :]
    return Q


# E[resid_g resid_g'] for resid = basis - c - beta*silu, x ~ N(0,1).
# (filled in below by calibration; placeholder identity keeps GPTQ sane
# if calibration is skipped)
_RESID_GRAM = np.eye(G) * 0.05


def _host_prep(x, base_w, base_b, spline_w):
    x = np.asarray(x, dtype=np.float32)
    base_w = np.asarray(base_w, dtype=np.float32)
    base_b = np.asarray(base_b, dtype=np.float32)
    spline_w = np.asarray(spline_w, dtype=np.float32)

    x_flat = np.ascontiguousarray(x.reshape(TOK, IN_F)).astype(BFNP)

    # spline weights: k = g*IN + i  ->  [G*IN, OUT]
    W8 = _quantize_w(spline_w)  # [O, I, G] e4m3 (scaled by WSCALE)
    Wk_hi = W8.transpose(2, 1, 0).reshape(G * IN_F, OUT_F)

    def pack(Wm):  # [G*IN, OUT] -> [128, NPAIR, 2, OUT]
        return np.ascontiguousarray(
            Wm.reshape(NPAIR, 2, 128, OUT_F).transpose(2, 0, 1, 3))

    whi = pack(Wk_hi)

    ncov = len(LO_COVER)
    wlo = None
    if ncov:
        Wk = spline_w.transpose(2, 1, 0).reshape(G * IN_F, OUT_F).astype(np.float64)
        Wlo_full = ((Wk * WSCALE - Wk_hi.astype(np.float64))
                    .astype(np.float32).astype(F8NP))
        parts = []
        for g in sorted(LO_COVER):
            blk = Wlo_full[g * IN_F:(g + 1) * IN_F]  # [IN, OUT]
            parts.append(blk.reshape(UPG, 2, 128, OUT_F).transpose(2, 0, 1, 3))
        wlo = np.ascontiguousarray(np.concatenate(parts, axis=1))

    # base weights with beta-fold, 0.5 silu2 factor, and WSCALE
    if CENTER:
        V = np.einsum("g,oig->io", _BETA, spline_w.astype(np.float64))
    else:
        V = 0.0
    bw_eff = 0.5 * (base_w.T.astype(np.float64) + V) * WSCALE
    bw = np.ascontiguousarray(
        bw_eff.reshape(KB, 128, OUT_F).transpose(1, 0, 2)).astype(BFNP)

    if CENTER:
        bias = (_C[None, :] * spline_w.astype(np.float64).sum(axis=1)).sum(axis=1)
    else:
        bias = np.zeros(OUT_F)
    bias = bias + base_b.astype(np.float64)

    in_maps = []
    for c in range(NCORES):
        shard = x_flat[c * TCORE:(c + 1) * TCORE, :]
        xT = shard.T  # [in, tok]
        xg = np.ascontiguousarray(
            xT.reshape(KB, 128, NG, GTOK).transpose(2, 1, 0, 3))
        m = {"xg": xg, "whi": whi, "basew": bw}
        if ncov:
            m["wlo"] = wlo
        in_maps.append(m)
    return in_maps, bias


def kernel(x, base_w, base_b, spline_w):
    global _NC_CACHE, LAST_RESULT
    from concourse.bass_utils import run_bass_kernel_spmd

    in_maps, bias = _host_prep(x, base_w, base_b, spline_w)
    if _NC_CACHE is None:
        _NC_CACHE = build_nc()
    res = run_bass_kernel_spmd(
        _NC_CACHE, in_maps, core_ids=list(range(NCORES)), trace=TRACE
    )
    LAST_RESULT = res
    outs = [np.asarray(r["out"], dtype=np.float64) for r in res.results]
    full = np.concatenate(outs, axis=0) / WSCALE + bias
    return full.astype(np.float32).reshape(4, 2048, OUT_F)


# revision 19
# speedup vs baseline: 1.0808x; 1.0174x over previous
"""KANLinear (RBF-KAN) Trainium2 kernel — fp8 DoubleRow version.

Math (matches the reference):
  x_flat [B=8192, IN=1024]
  base   = silu(x) @ base_w.T + base_b
  basis[b,i,g] = exp(-(d*(x[b,i]-grid[g]))**2),  grid = linspace(-2,2,8)
  spline = einsum('big,oig->bo', basis, spline_w)
  out    = base + spline

Implementation (data-parallel over tokens, 8 cores x 1024 tokens):
  - The spline contraction runs in fp8 e4m3 with perf_mode=DoubleRow:
    each MM contracts a PAIR of 128-row k-subtiles in the time a bf16
    MM contracts one (the PE moving port is byte-bound at 2B/cycle, and
    fp8 carries 2x K per byte). Spline MMs: 32 pairs x 8 m x 2 n = 512.
  - Accuracy (plain e4m3 is ~2.7% rel err, gate 2e-2):
      1. Variance reduction: quantize the RESIDUAL basis
         resid = basis - c_g - beta_g*silu(x); (c_g, beta_g) are
         least-squares fits (hardcoded). c folds into a host bias,
         beta folds into the base weights. |resid| ~ 0.68 |basis|.
      2. W-side: error-feedback (GPTQ-style) e4m3 rounding of W across
         the 8 correlated g-rows (host-side, uses the shared 8x8 resid
         Gram), plus optional explicit W_lo correction matmuls for the
         highest-variance grid points (LO_COVER knob).
  - Per core: 4 groups x 256 tokens; PSUM holds 2 groups -> seamless
    group overlap. basis via one DVE stt (v=(x-2g)*x) + one ACT Exp;
    silu2 = x*(1+tanh(x/2)); everything stays in the exp/tanh table.
  - base_b and the c-fold bias are added on the host; outputs are
    accumulated at WSCALE and divided on the host.
"""

import os
import sys

os.environ.setdefault("MYCRO_LOCAL_CACHE", "1")
for _p in ("/opt/trn_rl_repo", "/root/.axon_site/_ro/trn_rl_repo"):
    if os.path.isdir(_p) and _p not in sys.path:
        sys.path.insert(0, _p)

import numpy as np
import ml_dtypes

F8NP = ml_dtypes.float8_e4m3
BFNP = ml_dtypes.bfloat16

IN_F = 1024
OUT_F = 1024
G = 8
GRID_LO, GRID_HI = -2.0, 2.0
NCORES = 8
TOK = 8192
TCORE = TOK // NCORES     # 1024 tokens per core
NG = 4                    # token groups per core
GTOK = TCORE // NG        # 256 tokens per group
MT = GTOK // 128          # 2 psum m-tiles per group
KB = IN_F // 128          # 8 i-blocks
KS = G * KB               # 64 k-subtiles of 128
NPAIR = KS // 2           # 32 DoubleRow k-pairs
UPG = KB // 2             # 4 pairs per g-block

WSCALE = 64.0
_KDE = 1.1283791670955126  # 2/sqrt(pi): Derivative_Erf(t) = KDE * exp(-t^2)

_DELTA = float((GRID_HI - GRID_LO) / (G - 1))
_D = 1.0 / (_DELTA + 1e-6)
_D2 = _D * _D
_GRID = np.linspace(GRID_LO, GRID_HI, G, dtype=np.float32).astype(np.float64)

# Least-squares fit of basis_g(x) ~ c_g + beta_g*silu(x) over x ~ N(0,1)
# (bf16-rounded silu). Computed offline on the reference distribution.
_C = np.array([0.08754251, 0.20408037, 0.3485522, 0.42897628,
               0.37042523, 0.21574167, 0.07760693, 0.01135657])
_BETA = np.array([-0.09874898, -0.23330925, -0.36547238, -0.32523782,
                  -0.04140068, 0.27750214, 0.37861404, 0.270346])

CENTER = True
GPTQ_W = False  # measured: no gain (resid columns ~uncorrelated across g)
LO_COVER = (3, 4)         # grid points with explicit W_lo correction MMs

TRACE = False
LAST_RESULT = None
_NC_CACHE = None


def build_nc():
    from concourse import bacc
    import concourse.mybir as mybir
    import concourse.tile as tile

    F32 = mybir.dt.float32
    BF16 = mybir.dt.bfloat16
    F8 = mybir.dt.float8e4
    Alu = mybir.AluOpType
    Act = mybir.ActivationFunctionType
    DR = mybir.MatmulPerfMode.DoubleRow

    ncov = len(LO_COVER)
    nc = bacc.Bacc("TRN2", target_bir_lowering=False)
    xg_d = nc.dram_tensor("xg", [NG, 128, KB, GTOK], BF16, kind="ExternalInput")
    whi_d = nc.dram_tensor("whi", [128, NPAIR, 2, OUT_F], F8, kind="ExternalInput")
    bw_d = nc.dram_tensor("basew", [128, KB, OUT_F], BF16, kind="ExternalInput")
    out_d = nc.dram_tensor("out", [TCORE, OUT_F], F32, kind="ExternalOutput")
    if ncov:
        wlo_d = nc.dram_tensor("wlo", [128, ncov * UPG, 2, OUT_F], F8,
                               kind="ExternalInput")

    def exp_bias(g):
        gval = float(_GRID[g])
        return float(-_D2 * gval * gval)

    # activation() requires pre-registered [128,1] const APs for fp biases
    def register_const_ap(value):
        t = nc.alloc_sbuf_tensor(f"const-bias-{value}", [128, 1], F32)
        nc.gpsimd.memset(t.ap(), value)
        nc.const_aps.aps[(F32, value)] = t.ap()

    need = {float(-_D * _GRID[g]) for g in range(G)}  # DerivErf biases
    for value in sorted(need):
        register_const_ap(value)
    nc.all_engine_barrier()

    with tile.TileContext(nc) as tc:
        with (
            tc.tile_pool(name="const", bufs=1) as cpool,
            tc.tile_pool(name="xg", bufs=4) as xpool,
            tc.tile_pool(name="silu", bufs=4) as spool,
            tc.tile_pool(name="vsc", bufs=2) as vpool,
            tc.tile_pool(name="bsc", bufs=2) as bpool,
            tc.tile_pool(name="r1s", bufs=2) as rpool,
            tc.tile_pool(name="resid", bufs=8) as fpool,
            tc.tile_pool(name="osb", bufs=1) as opool,
            tc.tile_pool(name="psum", bufs=4, space="PSUM") as ppool,
        ):
            whi_sb = cpool.tile([128, NPAIR, 2, OUT_F], F8)
            if ncov:
                wlo_sb = cpool.tile([128, ncov * UPG, 2, OUT_F], F8)
            bw_sb = cpool.tile([128, KB, OUT_F], BF16)
            ones_sb = cpool.tile([1, 128], BF16)

            pending = []

            def emit_evictions():
                for ps_t, row in pending:
                    o = opool.tile([128, OUT_F], F32, tag="osb", name=f"o_{row}")
                    nc.vector.tensor_copy(o[:, 0:512], ps_t[:, 0:512])
                    nc.scalar.copy(o[:, 512:1024], ps_t[:, 512:1024])
                    nc.sync.dma_start(out_d[row * 128:(row + 1) * 128, :], o[:])
                pending.clear()

            xg_t = {}
            silu2_t = {}
            resid_t = {}
            ps_t = {}

            def emit_xg_dma(grp, halves=1):
                xg = xpool.tile([128, KB, GTOK], BF16, tag="xg", name=f"xg{grp}")
                xg_t[grp] = xg
                if halves == 1:
                    nc.sync.dma_start(xg[:], xg_d[grp])
                else:
                    h = KB // halves
                    for i in range(halves):
                        nc.sync.dma_start(xg[:, i * h:(i + 1) * h, :],
                                          xg_d[grp, :, i * h:(i + 1) * h, :])

            def emit_silu(grp, halves=1):
                xg = xg_t[grp]
                silu2 = spool.tile([128, KB, GTOK], BF16, tag="silu",
                                   name=f"s2{grp}")
                silu2_t[grp] = silu2
                h = KB // halves
                for i in range(halves):
                    xpart = xg[:, i * h:(i + 1) * h, :].rearrange(
                        "p k t -> p (k t)")
                    th = vpool.tile([128, h * GTOK], F32, tag="vsc",
                                    name=f"th{grp}_{i}")
                    nc.scalar.activation(th[:], xpart, Act.Tanh, scale=0.5)
                    nc.vector.scalar_tensor_tensor(
                        silu2[:, i * h:(i + 1) * h, :].rearrange(
                            "p k t -> p (k t)"),
                        th[:], 1.0, xpart, op0=Alu.add, op1=Alu.mult,
                    )

            def emit_chain(grp, g):
                # basis = KDE*exp(-(d x - d g)^2) in ONE ACT op
                # (Derivative_Erf); KDE is divided out of W on the host.
                xflat = xg_t[grp][:].rearrange("p k t -> p (k t)")
                resid = fpool.tile([128, KB, GTOK], F8, tag="resid",
                                   name=f"r{grp}_{g}")
                resid_t[(grp, g)] = resid
                rflat = resid[:].rearrange("p k t -> p (k t)")
                basis = bpool.tile([128, KB * GTOK], F32, tag="bsc",
                                   name=f"b{grp}_{g}")
                nc.scalar.activation(
                    basis[:], xflat, Act.Derivative_Erf,
                    bias=float(-_D * _GRID[g]), scale=float(_D),
                )
                if CENTER:
                    # bs = KDE*((beta/2)*silu2 + c)   (Pool ts, 1-input)
                    bs = rpool.tile([128, KB * GTOK], BF16, tag="r1s",
                                    name=f"bs_{grp}_{g}")
                    nc.gpsimd.tensor_scalar(
                        bs[:], silu2_t[grp][:].rearrange("p k t -> p (k t)"),
                        float(_KDE * _BETA[g] / 2.0), float(_KDE * _C[g]),
                        op0=Alu.mult, op1=Alu.add,
                    )
                    # resid8 = basis - bs, cast e4m3 (DVE tt)
                    nc.vector.tensor_tensor(
                        rflat, basis[:], bs[:], op=Alu.subtract,
                    )
                else:
                    nc.vector.tensor_copy(rflat, basis[:])

            def emit_base(grp):
                ps = [
                    ppool.tile([128, OUT_F], F32, tag="ps",
                               name=f"ps_g{grp}m{m}")
                    for m in range(MT)
                ]
                ps_t[grp] = ps
                silu2 = silu2_t[grp]
                for m in range(MT):
                    for kb in range(KB):
                        lhsT = silu2[:, kb, m * 128:(m + 1) * 128]
                        for n in range(2):
                            nc.tensor.matmul(
                                ps[m][:, n * 512:(n + 1) * 512],
                                lhsT,
                                bw_sb[:, kb, n * 512:(n + 1) * 512],
                                start=(kb == 0), stop=False,
                            )

            def emit_spline(grp, g):
                ps = ps_t[grp]
                resid = resid_t.pop((grp, g))
                covered = g in LO_COVER
                cov_idx = sum(1 for c in LO_COVER if c < g)
                last_g = g == G - 1
                for u in range(UPG):
                    t = g * UPG + u
                    for m in range(MT):
                        lhsT = resid[:, 2 * u:2 * u + 2, m * 128:(m + 1) * 128]
                        for n in range(2):
                            last_mm = last_g and u == UPG - 1 and not covered
                            nc.tensor.matmul(
                                ps[m][:, n * 512:(n + 1) * 512],
                                lhsT,
                                whi_sb[:, t, :, n * 512:(n + 1) * 512],
                                start=False,
                                stop=last_mm,
                                perf_mode=DR,
                            )
                            if covered:
                                nc.tensor.matmul(
                                    ps[m][:, n * 512:(n + 1) * 512],
                                    lhsT,
                                    wlo_sb[:, cov_idx * UPG + u, :,
                                           n * 512:(n + 1) * 512],
                                    start=False,
                                    stop=(last_g and u == UPG - 1),
                                    perf_mode=DR,
                                )
                if last_g:
                    for m in range(MT):
                        pending.append((ps[m], grp * MT + m))
                if g == 1 and pending:
                    emit_evictions()

            # ---- chain-unit and mm-unit streams, chain runs LOOKAHEAD ahead
            # silu(grp+1) is hoisted to mid-grp so the next group's base
            # matmuls never wait on the ACT backlog.
            chain_units = []
            for grp in range(NG):
                chain_units.append(lambda grp=grp: emit_silu(grp))
            for grp in range(NG):
                for g in range(G):
                    chain_units.append(lambda grp=grp, g=g: emit_chain(grp, g))

            mm_units = []
            for grp in range(NG):
                mm_units.append(lambda grp=grp: emit_base(grp))
                for g in range(G):
                    mm_units.append(lambda grp=grp, g=g: emit_spline(grp, g))

            # prologue: warmup junk MMs (HAM) + initial DMAs
            nc.vector.memset(ones_sb[:], 1.0)
            warm_ps = ppool.tile([128, OUT_F], F32, tag="ps", name="warm")
            for w in range(32):
                nc.tensor.matmul(
                    warm_ps[:, 0:128], ones_sb[0:1, :], ones_sb[0:1, :],
                    start=True, stop=True,
                )
            for grp in range(NG):
                emit_xg_dma(grp)
            nc.sync.dma_start(bw_sb[:], bw_d[:])
            for t in range(4):
                nc.sync.dma_start(whi_sb[:, t], whi_d[:, t])
            if ncov:
                nc.sync.dma_start(wlo_sb[:], wlo_d[:])
            for t in range(4, NPAIR):
                nc.sync.dma_start(whi_sb[:, t], whi_d[:, t])

            LOOKAHEAD = 4
            ci = 0
            for j, mm in enumerate(mm_units):
                target = min(len(chain_units),
                             (j * len(chain_units)) // len(mm_units) + LOOKAHEAD)
                while ci < target:
                    chain_units[ci]()
                    ci += 1
                mm()
            while ci < len(chain_units):
                chain_units[ci]()
                ci += 1
            emit_evictions()

    nc.compile()
    return nc


def _quantize_w(spline_w):
    """e4m3 quantization of W*WSCALE with optional GPTQ-style error
    feedback across the g dimension (shared Gram, hardcoded)."""
    W = spline_w.astype(np.float64) * WSCALE  # [O, I, G]
    if not GPTQ_W:
        Whi = (W.astype(np.float32)).astype(F8NP)
        return Whi
    # Shared 8x8 Gram of the centered residual basis under x~N(0,1).
    # E[resid_g resid_g'] computed offline on the reference distribution.
    H = _RESID_GRAM + 1e-6 * np.trace(_RESID_GRAM) / G * np.eye(G)
    Hinv = np.linalg.inv(H)
    U = np.linalg.cholesky(Hinv[::-1, ::-1].T)[::-1, ::-1].T
    Wp = W.copy()
    Q = np.zeros(W.shape, dtype=F8NP)
    for k in range(G):
        Q[:, :, k] = Wp[:, :, k].astype(np.float32).astype(F8NP)
        err = Wp[:, :, k] - Q[:, :, k].astype(np.float64)
        if k + 1 < G:
            coef = U[k, k + 1:] / U[k, k]
            Wp[:, :, k + 1:] -= err[:, :, None] * coef[None, None, :]
    return Q


# E[resid_g resid_g'] for resid = basis - c - beta*silu, x ~ N(0,1).
# (filled in below by calibration; placeholder identity keeps GPTQ sane
# if calibration is skipped)
_RESID_GRAM = np.eye(G) * 0.05


def _host_prep(x, base_w, base_b, spline_w):
    x = np.asarray(x, dtype=np.float32)
    base_w = np.asarray(base_w, dtype=np.float32)
    base_b = np.asarray(base_b, dtype=np.float32)
    spline_w = np.asarray(spline_w, dtype=np.float32)

    x_flat = np.ascontiguousarray(x.reshape(TOK, IN_F)).astype(BFNP)

    # spline weights: k = g*IN + i  ->  [G*IN, OUT]
    # device resid is scaled by KDE (Derivative_Erf); divide W to compensate
    W8 = _quantize_w(spline_w / _KDE)  # [O, I, G] e4m3 (scaled by WSCALE)
    Wk_hi = W8.transpose(2, 1, 0).reshape(G * IN_F, OUT_F)

    def pack(Wm):  # [G*IN, OUT] -> [128, NPAIR, 2, OUT]
        return np.ascontiguousarray(
            Wm.reshape(NPAIR, 2, 128, OUT_F).transpose(2, 0, 1, 3))

    whi = pack(Wk_hi)

    ncov = len(LO_COVER)
    wlo = None
    if ncov:
        Wk = (spline_w / _KDE).transpose(2, 1, 0).reshape(
            G * IN_F, OUT_F).astype(np.float64)
        Wlo_full = ((Wk * WSCALE - Wk_hi.astype(np.float64))
                    .astype(np.float32).astype(F8NP))
        parts = []
        for g in sorted(LO_COVER):
            blk = Wlo_full[g * IN_F:(g + 1) * IN_F]  # [IN, OUT]
            parts.append(blk.reshape(UPG, 2, 128, OUT_F).transpose(2, 0, 1, 3))
        wlo = np.ascontiguousarray(np.concatenate(parts, axis=1))

    # base weights with beta-fold, 0.5 silu2 factor, and WSCALE
    if CENTER:
        V = np.einsum("g,oig->io", _BETA, spline_w.astype(np.float64))
    else:
        V = 0.0
    bw_eff = 0.5 * (base_w.T.astype(np.float64) + V) * WSCALE
    bw = np.ascontiguousarray(
        bw_eff.reshape(KB, 128, OUT_F).transpose(1, 0, 2)).astype(BFNP)

    if CENTER:
        bias = (_C[None, :] * spline_w.astype(np.float64).sum(axis=1)).sum(axis=1)
    else:
        bias = np.zeros(OUT_F)
    bias = bias + base_b.astype(np.float64)

    in_maps = []
    for c in range(NCORES):
        shard = x_flat[c * TCORE:(c + 1) * TCORE, :]
        xT = shard.T  # [in, tok]
        xg = np.ascontiguousarray(
            xT.reshape(KB, 128, NG, GTOK).transpose(2, 1, 0, 3))
        m = {"xg": xg, "whi": whi, "basew": bw}
        if ncov:
            m["wlo"] = wlo
        in_maps.append(m)
    return in_maps, bias


def kernel(x, base_w, base_b, spline_w):
    global _NC_CACHE, LAST_RESULT
    from concourse.bass_utils import run_bass_kernel_spmd

    in_maps, bias = _host_prep(x, base_w, base_b, spline_w)
    if _NC_CACHE is None:
        _NC_CACHE = build_nc()
    res = run_bass_kernel_spmd(
        _NC_CACHE, in_maps, core_ids=list(range(NCORES)), trace=TRACE
    )
    LAST_RESULT = res
    outs = [np.asarray(r["out"], dtype=np.float64) for r in res.results]
    full = np.concatenate(outs, axis=0) / WSCALE + bias
    return full.astype(np.float32).reshape(4, 2048, OUT_F)


# revision 20
# speedup vs baseline: 1.0823x; 1.0014x over previous
"""KANLinear (RBF-KAN) Trainium2 kernel — fp8 DoubleRow version.

Math (matches the reference):
  x_flat [B=8192, IN=1024]
  base   = silu(x) @ base_w.T + base_b
  basis[b,i,g] = exp(-(d*(x[b,i]-grid[g]))**2),  grid = linspace(-2,2,8)
  spline = einsum('big,oig->bo', basis, spline_w)
  out    = base + spline

Implementation (data-parallel over tokens, 8 cores x 1024 tokens):
  - The spline contraction runs in fp8 e4m3 with perf_mode=DoubleRow:
    each MM contracts a PAIR of 128-row k-subtiles in the time a bf16
    MM contracts one (the PE moving port is byte-bound at 2B/cycle, and
    fp8 carries 2x K per byte). Spline MMs: 32 pairs x 8 m x 2 n = 512.
  - Accuracy (plain e4m3 is ~2.7% rel err, gate 2e-2):
      1. Variance reduction: quantize the RESIDUAL basis
         resid = basis - c_g - beta_g*silu(x); (c_g, beta_g) are
         least-squares fits (hardcoded). c folds into a host bias,
         beta folds into the base weights. |resid| ~ 0.68 |basis|.
      2. W-side: error-feedback (GPTQ-style) e4m3 rounding of W across
         the 8 correlated g-rows (host-side, uses the shared 8x8 resid
         Gram), plus optional explicit W_lo correction matmuls for the
         highest-variance grid points (LO_COVER knob).
  - Per core: 4 groups x 256 tokens; PSUM holds 2 groups -> seamless
    group overlap. basis via one DVE stt (v=(x-2g)*x) + one ACT Exp;
    silu2 = x*(1+tanh(x/2)); everything stays in the exp/tanh table.
  - base_b and the c-fold bias are added on the host; outputs are
    accumulated at WSCALE and divided on the host.
"""

import os
import sys

os.environ.setdefault("MYCRO_LOCAL_CACHE", "1")
for _p in ("/opt/trn_rl_repo", "/root/.axon_site/_ro/trn_rl_repo"):
    if os.path.isdir(_p) and _p not in sys.path:
        sys.path.insert(0, _p)

import numpy as np
import ml_dtypes

F8NP = ml_dtypes.float8_e4m3
BFNP = ml_dtypes.bfloat16

IN_F = 1024
OUT_F = 1024
G = 8
GRID_LO, GRID_HI = -2.0, 2.0
NCORES = 8
TOK = 8192
TCORE = TOK // NCORES     # 1024 tokens per core
NG = 4                    # token groups per core
GTOK = TCORE // NG        # 256 tokens per group
MT = GTOK // 128          # 2 psum m-tiles per group
KB = IN_F // 128          # 8 i-blocks
KS = G * KB               # 64 k-subtiles of 128
NPAIR = KS // 2           # 32 DoubleRow k-pairs
UPG = KB // 2             # 4 pairs per g-block

WSCALE = 64.0
_KDE = 1.1283791670955126  # 2/sqrt(pi): Derivative_Erf(t) = KDE * exp(-t^2)

_DELTA = float((GRID_HI - GRID_LO) / (G - 1))
_D = 1.0 / (_DELTA + 1e-6)
_D2 = _D * _D
_GRID = np.linspace(GRID_LO, GRID_HI, G, dtype=np.float32).astype(np.float64)

# Least-squares fit of basis_g(x) ~ c_g + beta_g*silu(x) over x ~ N(0,1)
# (bf16-rounded silu). Computed offline on the reference distribution.
_C = np.array([0.08754251, 0.20408037, 0.3485522, 0.42897628,
               0.37042523, 0.21574167, 0.07760693, 0.01135657])
_BETA = np.array([-0.09874898, -0.23330925, -0.36547238, -0.32523782,
                  -0.04140068, 0.27750214, 0.37861404, 0.270346])

CENTER = True
GPTQ_W = False  # measured: no gain (resid columns ~uncorrelated across g)
LO_COVER = (3, 4)         # grid points with explicit W_lo correction MMs

TRACE = False
LAST_RESULT = None
_NC_CACHE = None


def build_nc():
    from concourse import bacc
    import concourse.mybir as mybir
    import concourse.tile as tile

    F32 = mybir.dt.float32
    BF16 = mybir.dt.bfloat16
    F8 = mybir.dt.float8e4
    Alu = mybir.AluOpType
    Act = mybir.ActivationFunctionType
    DR = mybir.MatmulPerfMode.DoubleRow

    ncov = len(LO_COVER)
    nc = bacc.Bacc("TRN2", target_bir_lowering=False)
    xg_d = nc.dram_tensor("xg", [NG, 128, KB, GTOK], BF16, kind="ExternalInput")
    whi_d = nc.dram_tensor("whi", [128, NPAIR, 2, OUT_F], F8, kind="ExternalInput")
    bw_d = nc.dram_tensor("basew", [128, KB, OUT_F], BF16, kind="ExternalInput")
    out_d = nc.dram_tensor("out", [TCORE, OUT_F], F32, kind="ExternalOutput")
    if ncov:
        wlo_d = nc.dram_tensor("wlo", [128, ncov * UPG, 2, OUT_F], F8,
                               kind="ExternalInput")

    def exp_bias(g):
        gval = float(_GRID[g])
        return float(-_D2 * gval * gval)

    # activation() requires pre-registered [128,1] const APs for fp biases
    def register_const_ap(value):
        t = nc.alloc_sbuf_tensor(f"const-bias-{value}", [128, 1], F32)
        nc.gpsimd.memset(t.ap(), value)
        nc.const_aps.aps[(F32, value)] = t.ap()

    need = {float(-_D * _GRID[g]) for g in range(G)}  # DerivErf biases
    for value in sorted(need):
        register_const_ap(value)
    nc.all_engine_barrier()

    with tile.TileContext(nc) as tc:
        with (
            tc.tile_pool(name="const", bufs=1) as cpool,
            tc.tile_pool(name="xg", bufs=4) as xpool,
            tc.tile_pool(name="silu", bufs=4) as spool,
            tc.tile_pool(name="vsc", bufs=2) as vpool,
            tc.tile_pool(name="bsc", bufs=2) as bpool,
            tc.tile_pool(name="r1s", bufs=2) as rpool,
            tc.tile_pool(name="resid", bufs=8) as fpool,
            tc.tile_pool(name="osb", bufs=1) as opool,
            tc.tile_pool(name="psum", bufs=4, space="PSUM") as ppool,
        ):
            whi_sb = cpool.tile([128, NPAIR, 2, OUT_F], F8)
            if ncov:
                wlo_sb = cpool.tile([128, ncov * UPG, 2, OUT_F], F8)
            bw_sb = cpool.tile([128, KB, OUT_F], BF16)
            ones_sb = cpool.tile([1, 128], BF16)

            pending = []

            def emit_evictions():
                for ps_t, row in pending:
                    o = opool.tile([128, OUT_F], F32, tag="osb", name=f"o_{row}")
                    nc.vector.tensor_copy(o[:, 0:512], ps_t[:, 0:512])
                    nc.scalar.copy(o[:, 512:1024], ps_t[:, 512:1024])
                    nc.sync.dma_start(out_d[row * 128:(row + 1) * 128, :], o[:])
                pending.clear()

            xg_t = {}
            silu2_t = {}
            resid_t = {}
            ps_t = {}

            def emit_xg_dma(grp, halves=1):
                xg = xpool.tile([128, KB, GTOK], BF16, tag="xg", name=f"xg{grp}")
                xg_t[grp] = xg
                if halves == 1:
                    nc.sync.dma_start(xg[:], xg_d[grp])
                else:
                    h = KB // halves
                    for i in range(halves):
                        nc.sync.dma_start(xg[:, i * h:(i + 1) * h, :],
                                          xg_d[grp, :, i * h:(i + 1) * h, :])

            def emit_silu(grp, halves=1):
                xg = xg_t[grp]
                silu2 = spool.tile([128, KB, GTOK], BF16, tag="silu",
                                   name=f"s2{grp}")
                silu2_t[grp] = silu2
                h = KB // halves
                for i in range(halves):
                    xpart = xg[:, i * h:(i + 1) * h, :].rearrange(
                        "p k t -> p (k t)")
                    th = vpool.tile([128, h * GTOK], F32, tag="vsc",
                                    name=f"th{grp}_{i}")
                    nc.scalar.activation(th[:], xpart, Act.Tanh, scale=0.5)
                    nc.vector.scalar_tensor_tensor(
                        silu2[:, i * h:(i + 1) * h, :].rearrange(
                            "p k t -> p (k t)"),
                        th[:], 1.0, xpart, op0=Alu.add, op1=Alu.mult,
                    )

            def emit_chain(grp, g):
                # basis = KDE*exp(-(d x - d g)^2) in ONE ACT op
                # (Derivative_Erf); KDE is divided out of W on the host.
                xflat = xg_t[grp][:].rearrange("p k t -> p (k t)")
                resid = fpool.tile([128, KB, GTOK], F8, tag="resid",
                                   name=f"r{grp}_{g}")
                resid_t[(grp, g)] = resid
                rflat = resid[:].rearrange("p k t -> p (k t)")
                basis = bpool.tile([128, KB * GTOK], F32, tag="bsc",
                                   name=f"b{grp}_{g}")
                nc.scalar.activation(
                    basis[:], xflat, Act.Derivative_Erf,
                    bias=float(-_D * _GRID[g]), scale=float(_D),
                )
                if CENTER:
                    # bs = KDE*((beta/2)*silu2 + c)   (Pool ts, 1-input)
                    bs = rpool.tile([128, KB * GTOK], BF16, tag="r1s",
                                    name=f"bs_{grp}_{g}")
                    nc.gpsimd.tensor_scalar(
                        bs[:], silu2_t[grp][:].rearrange("p k t -> p (k t)"),
                        float(_KDE * _BETA[g] / 2.0), float(_KDE * _C[g]),
                        op0=Alu.mult, op1=Alu.add,
                    )
                    # resid8 = basis - bs, cast e4m3 (DVE tt)
                    nc.vector.tensor_tensor(
                        rflat, basis[:], bs[:], op=Alu.subtract,
                    )
                else:
                    nc.vector.tensor_copy(rflat, basis[:])

            def emit_base(grp):
                ps = [
                    ppool.tile([128, OUT_F], F32, tag="ps",
                               name=f"ps_g{grp}m{m}")
                    for m in range(MT)
                ]
                ps_t[grp] = ps
                silu2 = silu2_t[grp]
                for m in range(MT):
                    for kb in range(KB):
                        lhsT = silu2[:, kb, m * 128:(m + 1) * 128]
                        for n in range(2):
                            nc.tensor.matmul(
                                ps[m][:, n * 512:(n + 1) * 512],
                                lhsT,
                                bw_sb[:, kb, n * 512:(n + 1) * 512],
                                start=(kb == 0), stop=False,
                            )

            def emit_spline(grp, g):
                ps = ps_t[grp]
                resid = resid_t.pop((grp, g))
                covered = g in LO_COVER
                cov_idx = sum(1 for c in LO_COVER if c < g)
                last_g = g == G - 1
                # last g-block of each group goes m-major so m0's psum can
                # evict while m1's matmuls still stream
                m_outer = last_g
                order = ([(m, u) for m in range(MT) for u in range(UPG)]
                         if m_outer else
                         [(m, u) for u in range(UPG) for m in range(MT)])
                for m, u in order:
                    t = g * UPG + u
                    lhsT = resid[:, 2 * u:2 * u + 2, m * 128:(m + 1) * 128]
                    for n in range(2):
                        last_mm = last_g and u == UPG - 1 and not covered
                        nc.tensor.matmul(
                            ps[m][:, n * 512:(n + 1) * 512],
                            lhsT,
                            whi_sb[:, t, :, n * 512:(n + 1) * 512],
                            start=False,
                            stop=last_mm,
                            perf_mode=DR,
                        )
                        if covered:
                            nc.tensor.matmul(
                                ps[m][:, n * 512:(n + 1) * 512],
                                lhsT,
                                wlo_sb[:, cov_idx * UPG + u, :,
                                       n * 512:(n + 1) * 512],
                                start=False,
                                stop=(last_g and u == UPG - 1),
                                perf_mode=DR,
                            )
                    if last_g and u == UPG - 1:
                        pending.append((ps[m], grp * MT + m))
                        if grp == NG - 1:
                            emit_evictions()
                if g == 1 and pending:
                    emit_evictions()

            # ---- chain-unit and mm-unit streams, chain runs LOOKAHEAD ahead
            # silu(grp+1) is hoisted to mid-grp so the next group's base
            # matmuls never wait on the ACT backlog.
            chain_units = []
            for grp in range(NG):
                chain_units.append(lambda grp=grp: emit_silu(grp))
            for grp in range(NG):
                for g in range(G):
                    chain_units.append(lambda grp=grp, g=g: emit_chain(grp, g))

            mm_units = []
            for grp in range(NG):
                mm_units.append(lambda grp=grp: emit_base(grp))
                for g in range(G):
                    mm_units.append(lambda grp=grp, g=g: emit_spline(grp, g))

            # prologue: warmup junk MMs (HAM) + initial DMAs
            nc.vector.memset(ones_sb[:], 1.0)
            warm_ps = ppool.tile([128, OUT_F], F32, tag="ps", name="warm")
            for w in range(32):
                nc.tensor.matmul(
                    warm_ps[:, 0:128], ones_sb[0:1, :], ones_sb[0:1, :],
                    start=True, stop=True,
                )
            nc.sync.dma_start(bw_sb[:], bw_d[:])
            for grp in range(NG):
                emit_xg_dma(grp)
            for t in range(4):
                nc.sync.dma_start(whi_sb[:, t], whi_d[:, t])
            if ncov:
                nc.sync.dma_start(wlo_sb[:], wlo_d[:])
            for t in range(4, NPAIR):
                nc.sync.dma_start(whi_sb[:, t], whi_d[:, t])

            LOOKAHEAD = 4
            ci = 0
            for j, mm in enumerate(mm_units):
                target = min(len(chain_units),
                             (j * len(chain_units)) // len(mm_units) + LOOKAHEAD)
                while ci < target:
                    chain_units[ci]()
                    ci += 1
                mm()
            while ci < len(chain_units):
                chain_units[ci]()
                ci += 1
            emit_evictions()

    nc.compile()
    return nc


def _quantize_w(spline_w):
    """e4m3 quantization of W*WSCALE with optional GPTQ-style error
    feedback across the g dimension (shared Gram, hardcoded)."""
    W = spline_w.astype(np.float64) * WSCALE  # [O, I, G]
    if not GPTQ_W:
        Whi = (W.astype(np.float32)).astype(F8NP)
        return Whi
    # Shared 8x8 Gram of the centered residual basis under x~N(0,1).
    # E[resid_g resid_g'] computed offline on the reference distribution.
    H = _RESID_GRAM + 1e-6 * np.trace(_RESID_GRAM) / G * np.eye(G)
    Hinv = np.linalg.inv(H)
    U = np.linalg.cholesky(Hinv[::-1, ::-1].T)[::-1, ::-1].T
    Wp = W.copy()
    Q = np.zeros(W.shape, dtype=F8NP)
    for k in range(G):
        Q[:, :, k] = Wp[:, :, k].astype(np.float32).astype(F8NP)
        err = Wp[:, :, k] - Q[:, :, k].astype(np.float64)
        if k + 1 < G:
            coef = U[k, k + 1:] / U[k, k]
            Wp[:, :, k + 1:] -= err[:, :, None] * coef[None, None, :]
    return Q


# E[resid_g resid_g'] for resid = basis - c - beta*silu, x ~ N(0,1).
# (filled in below by calibration; placeholder identity keeps GPTQ sane
# if calibration is skipped)
_RESID_GRAM = np.eye(G) * 0.05


def _host_prep(x, base_w, base_b, spline_w):
    x = np.asarray(x, dtype=np.float32)
    base_w = np.asarray(base_w, dtype=np.float32)
    base_b = np.asarray(base_b, dtype=np.float32)
    spline_w = np.asarray(spline_w, dtype=np.float32)

    x_flat = np.ascontiguousarray(x.reshape(TOK, IN_F)).astype(BFNP)

    # spline weights: k = g*IN + i  ->  [G*IN, OUT]
    # device resid is scaled by KDE (Derivative_Erf); divide W to compensate
    W8 = _quantize_w(spline_w / _KDE)  # [O, I, G] e4m3 (scaled by WSCALE)
    Wk_hi = W8.transpose(2, 1, 0).reshape(G * IN_F, OUT_F)

    def pack(Wm):  # [G*IN, OUT] -> [128, NPAIR, 2, OUT]
        return np.ascontiguousarray(
            Wm.reshape(NPAIR, 2, 128, OUT_F).transpose(2, 0, 1, 3))

    whi = pack(Wk_hi)

    ncov = len(LO_COVER)
    wlo = None
    if ncov:
        Wk = (spline_w / _KDE).transpose(2, 1, 0).reshape(
            G * IN_F, OUT_F).astype(np.float64)
        Wlo_full = ((Wk * WSCALE - Wk_hi.astype(np.float64))
                    .astype(np.float32).astype(F8NP))
        parts = []
        for g in sorted(LO_COVER):
            blk = Wlo_full[g * IN_F:(g + 1) * IN_F]  # [IN, OUT]
            parts.append(blk.reshape(UPG, 2, 128, OUT_F).transpose(2, 0, 1, 3))
        wlo = np.ascontiguousarray(np.concatenate(parts, axis=1))

    # base weights with beta-fold, 0.5 silu2 factor, and WSCALE
    if CENTER:
        V = np.einsum("g,oig->io", _BETA, spline_w.astype(np.float64))
    else:
        V = 0.0
    bw_eff = 0.5 * (base_w.T.astype(np.float64) + V) * WSCALE
    bw = np.ascontiguousarray(
        bw_eff.reshape(KB, 128, OUT_F).transpose(1, 0, 2)).astype(BFNP)

    if CENTER:
        bias = (_C[None, :] * spline_w.astype(np.float64).sum(axis=1)).sum(axis=1)
    else:
        bias = np.zeros(OUT_F)
    bias = bias + base_b.astype(np.float64)

    in_maps = []
    for c in range(NCORES):
        shard = x_flat[c * TCORE:(c + 1) * TCORE, :]
        xT = shard.T  # [in, tok]
        xg = np.ascontiguousarray(
            xT.reshape(KB, 128, NG, GTOK).transpose(2, 1, 0, 3))
        m = {"xg": xg, "whi": whi, "basew": bw}
        if ncov:
            m["wlo"] = wlo
        in_maps.append(m)
    return in_maps, bias


def kernel(x, base_w, base_b, spline_w):
    global _NC_CACHE, LAST_RESULT
    from concourse.bass_utils import run_bass_kernel_spmd

    in_maps, bias = _host_prep(x, base_w, base_b, spline_w)
    if _NC_CACHE is None:
        _NC_CACHE = build_nc()
    res = run_bass_kernel_spmd(
        _NC_CACHE, in_maps, core_ids=list(range(NCORES)), trace=TRACE
    )
    LAST_RESULT = res
    outs = [np.asarray(r["out"], dtype=np.float64) for r in res.results]
    full = np.concatenate(outs, axis=0) / WSCALE + bias
    return full.astype(np.float32).reshape(4, 2048, OUT_F)


# revision 22
# speedup vs baseline: 1.1067x; 1.0226x over previous
"""KANLinear (RBF-KAN) Trainium2 kernel — fp8 DoubleRow version.

Math (matches the reference):
  x_flat [B=8192, IN=1024]
  base   = silu(x) @ base_w.T + base_b
  basis[b,i,g] = exp(-(d*(x[b,i]-grid[g]))**2),  grid = linspace(-2,2,8)
  spline = einsum('big,oig->bo', basis, spline_w)
  out    = base + spline

Implementation (data-parallel over tokens, 8 cores x 1024 tokens):
  - The spline contraction runs in fp8 e4m3 with perf_mode=DoubleRow:
    each MM contracts a PAIR of 128-row k-subtiles in the time a bf16
    MM contracts one (the PE moving port is byte-bound at 2B/cycle, and
    fp8 carries 2x K per byte). Spline MMs: 32 pairs x 8 m x 2 n = 512.
  - Accuracy (plain e4m3 is ~2.7% rel err, gate 2e-2):
      1. Variance reduction: quantize the RESIDUAL basis
         resid = basis - c_g - beta_g*silu(x); (c_g, beta_g) are
         least-squares fits (hardcoded). c folds into a host bias,
         beta folds into the base weights. |resid| ~ 0.68 |basis|.
      2. W-side: error-feedback (GPTQ-style) e4m3 rounding of W across
         the 8 correlated g-rows (host-side, uses the shared 8x8 resid
         Gram), plus optional explicit W_lo correction matmuls for the
         highest-variance grid points (LO_COVER knob).
  - Per core: 4 groups x 256 tokens; PSUM holds 2 groups -> seamless
    group overlap. basis via one DVE stt (v=(x-2g)*x) + one ACT Exp;
    silu2 = x*(1+tanh(x/2)); everything stays in the exp/tanh table.
  - base_b and the c-fold bias are added on the host; outputs are
    accumulated at WSCALE and divided on the host.
"""

import os
import sys

os.environ.setdefault("MYCRO_LOCAL_CACHE", "1")
for _p in ("/opt/trn_rl_repo", "/root/.axon_site/_ro/trn_rl_repo"):
    if os.path.isdir(_p) and _p not in sys.path:
        sys.path.insert(0, _p)

import numpy as np
import ml_dtypes

F8NP = ml_dtypes.float8_e4m3
BFNP = ml_dtypes.bfloat16

IN_F = 1024
OUT_F = 1024
G = 8
GRID_LO, GRID_HI = -2.0, 2.0
NCORES = 8
TOK = 8192
TCORE = TOK // NCORES     # 1024 tokens per core
NG = 4                    # token groups per core
GTOK = TCORE // NG        # 256 tokens per group
MT = GTOK // 128          # 2 psum m-tiles per group
KB = IN_F // 128          # 8 i-blocks
KS = G * KB               # 64 k-subtiles of 128
NPAIR = KS // 2           # 32 DoubleRow k-pairs
UPG = KB // 2             # 4 pairs per g-block

WSCALE = 64.0
_KDE = 1.1283791670955126  # 2/sqrt(pi): Derivative_Erf(t) = KDE * exp(-t^2)

_DELTA = float((GRID_HI - GRID_LO) / (G - 1))
_D = 1.0 / (_DELTA + 1e-6)
_D2 = _D * _D
_GRID = np.linspace(GRID_LO, GRID_HI, G, dtype=np.float32).astype(np.float64)

# Least-squares fit of basis_g(x) ~ c_g + beta_g*silu(x) over x ~ N(0,1)
# (bf16-rounded silu). Computed offline on the reference distribution.
_C = np.array([0.08754251, 0.20408037, 0.3485522, 0.42897628,
               0.37042523, 0.21574167, 0.07760693, 0.01135657])
_BETA = np.array([-0.09874898, -0.23330925, -0.36547238, -0.32523782,
                  -0.04140068, 0.27750214, 0.37861404, 0.270346])

CENTER = True
GPTQ_W = False  # measured: no gain (resid columns ~uncorrelated across g)
LO_COVER = (3, 4)         # grid points with explicit W_lo correction MMs

TRACE = False
LAST_RESULT = None
_NC_CACHE = None


def build_nc():
    from concourse import bacc
    import concourse.mybir as mybir
    import concourse.tile as tile
    from concourse.tile_rust import add_dep_helper

    F32 = mybir.dt.float32
    BF16 = mybir.dt.bfloat16
    F8 = mybir.dt.float8e4
    Alu = mybir.AluOpType
    Act = mybir.ActivationFunctionType
    DR = mybir.MatmulPerfMode.DoubleRow

    ncov = len(LO_COVER)
    nc = bacc.Bacc("TRN2", target_bir_lowering=False)
    xg_d = nc.dram_tensor("xg", [NG, 128, KB, GTOK], BF16, kind="ExternalInput")
    whi_d = nc.dram_tensor("whi", [128, NPAIR, 2, OUT_F], F8, kind="ExternalInput")
    bw_d = nc.dram_tensor("basew", [128, KB, OUT_F], BF16, kind="ExternalInput")
    out_d = nc.dram_tensor("out", [TCORE, OUT_F], F32, kind="ExternalOutput")
    if ncov:
        wlo_d = nc.dram_tensor("wlo", [128, ncov * UPG, 2, OUT_F], F8,
                               kind="ExternalInput")

    def exp_bias(g):
        gval = float(_GRID[g])
        return float(-_D2 * gval * gval)

    # activation() requires pre-registered [128,1] const APs for fp biases
    def register_const_ap(value):
        t = nc.alloc_sbuf_tensor(f"const-bias-{value}", [128, 1], F32)
        nc.gpsimd.memset(t.ap(), value)
        nc.const_aps.aps[(F32, value)] = t.ap()

    need = {float(-_D * _GRID[g]) for g in range(G)}  # DerivErf biases
    for value in sorted(need):
        register_const_ap(value)
    nc.all_engine_barrier()

    with tile.TileContext(nc) as tc:
        with (
            tc.tile_pool(name="const", bufs=1) as cpool,
            tc.tile_pool(name="xg", bufs=4) as xpool,
            tc.tile_pool(name="silu", bufs=4) as spool,
            tc.tile_pool(name="vsc", bufs=2) as vpool,
            tc.tile_pool(name="bsc", bufs=2) as bpool,
            tc.tile_pool(name="r1s", bufs=2) as rpool,
            tc.tile_pool(name="resid", bufs=8) as fpool,
            tc.tile_pool(name="osb", bufs=1) as opool,
            tc.tile_pool(name="psum", bufs=4, space="PSUM") as ppool,
        ):
            whi_sb = cpool.tile([128, NPAIR, 2, OUT_F], F8)
            if ncov:
                wlo_sb = cpool.tile([128, ncov * UPG, 2, OUT_F], F8)
            bw_sb = cpool.tile([128, KB, OUT_F], BF16)
            ones_sb = cpool.tile([1, 128], BF16)

            pending = []

            def emit_evictions():
                for ps_t, row in pending:
                    o = opool.tile([128, OUT_F], F32, tag="osb", name=f"o_{row}")
                    nc.vector.tensor_copy(o[:, 0:512], ps_t[:, 0:512])
                    nc.scalar.copy(o[:, 512:1024], ps_t[:, 512:1024])
                    nc.sync.dma_start(out_d[row * 128:(row + 1) * 128, :], o[:])
                pending.clear()

            xg_t = {}
            silu2_t = {}
            resid_t = {}
            ps_t = {}
            tanh_insts = []
            derf_first = [True]

            def emit_xg_dma(grp, halves=1):
                xg = xpool.tile([128, KB, GTOK], BF16, tag="xg", name=f"xg{grp}")
                xg_t[grp] = xg
                if halves == 1:
                    nc.sync.dma_start(xg[:], xg_d[grp])
                else:
                    h = KB // halves
                    for i in range(halves):
                        nc.sync.dma_start(xg[:, i * h:(i + 1) * h, :],
                                          xg_d[grp, :, i * h:(i + 1) * h, :])

            def emit_silu(grp, halves=1):
                xg = xg_t[grp]
                silu2 = spool.tile([128, KB, GTOK], BF16, tag="silu",
                                   name=f"s2{grp}")
                silu2_t[grp] = silu2
                h = KB // halves
                for i in range(halves):
                    xpart = xg[:, i * h:(i + 1) * h, :].rearrange(
                        "p k t -> p (k t)")
                    th = vpool.tile([128, h * GTOK], F32, tag="vsc",
                                    name=f"th{grp}_{i}")
                    tanh_insts.append(
                        nc.scalar.activation(th[:], xpart, Act.Tanh, scale=0.5))
                    nc.vector.scalar_tensor_tensor(
                        silu2[:, i * h:(i + 1) * h, :].rearrange(
                            "p k t -> p (k t)"),
                        th[:], 1.0, xpart, op0=Alu.add, op1=Alu.mult,
                    )

            def emit_chain(grp, g):
                # basis = KDE*exp(-(d x - d g)^2) in ONE ACT op
                # (Derivative_Erf); KDE is divided out of W on the host.
                xflat = xg_t[grp][:].rearrange("p k t -> p (k t)")
                resid = fpool.tile([128, KB, GTOK], F8, tag="resid",
                                   name=f"r{grp}_{g}")
                resid_t[(grp, g)] = resid
                rflat = resid[:].rearrange("p k t -> p (k t)")
                basis = bpool.tile([128, KB * GTOK], F32, tag="bsc",
                                   name=f"b{grp}_{g}")
                derf = nc.scalar.activation(
                    basis[:], xflat, Act.Derivative_Erf,
                    bias=float(-_D * _GRID[g]), scale=float(_D),
                )
                if derf_first[0] and tanh_insts:
                    add_dep_helper(derf.ins, tanh_insts[-1].ins, sync=False,
                                   reason="batch tanh before derf (act table)")
                    derf_first[0] = False
                if CENTER:
                    # bs = KDE*((beta/2)*silu2 + c)   (Pool ts, 1-input)
                    bs = rpool.tile([128, KB * GTOK], BF16, tag="r1s",
                                    name=f"bs_{grp}_{g}")
                    nc.gpsimd.tensor_scalar(
                        bs[:], silu2_t[grp][:].rearrange("p k t -> p (k t)"),
                        float(_KDE * _BETA[g] / 2.0), float(_KDE * _C[g]),
                        op0=Alu.mult, op1=Alu.add,
                    )
                    # resid8 = basis - bs, cast e4m3 (DVE tt)
                    nc.vector.tensor_tensor(
                        rflat, basis[:], bs[:], op=Alu.subtract,
                    )
                else:
                    nc.vector.tensor_copy(rflat, basis[:])

            def emit_base(grp):
                ps = [
                    ppool.tile([128, OUT_F], F32, tag="ps",
                               name=f"ps_g{grp}m{m}")
                    for m in range(MT)
                ]
                ps_t[grp] = ps
                silu2 = silu2_t[grp]
                for m in range(MT):
                    for kb in range(KB):
                        lhsT = silu2[:, kb, m * 128:(m + 1) * 128]
                        for n in range(2):
                            nc.tensor.matmul(
                                ps[m][:, n * 512:(n + 1) * 512],
                                lhsT,
                                bw_sb[:, kb, n * 512:(n + 1) * 512],
                                start=(kb == 0), stop=False,
                            )

            def emit_spline(grp, g):
                ps = ps_t[grp]
                resid = resid_t.pop((grp, g))
                covered = g in LO_COVER
                cov_idx = sum(1 for c in LO_COVER if c < g)
                last_g = g == G - 1
                # last g-block of each group goes m-major so m0's psum can
                # evict while m1's matmuls still stream
                m_outer = last_g
                order = ([(m, u) for m in range(MT) for u in range(UPG)]
                         if m_outer else
                         [(m, u) for u in range(UPG) for m in range(MT)])
                for m, u in order:
                    t = g * UPG + u
                    lhsT = resid[:, 2 * u:2 * u + 2, m * 128:(m + 1) * 128]
                    for n in range(2):
                        last_mm = last_g and u == UPG - 1 and not covered
                        nc.tensor.matmul(
                            ps[m][:, n * 512:(n + 1) * 512],
                            lhsT,
                            whi_sb[:, t, :, n * 512:(n + 1) * 512],
                            start=False,
                            stop=last_mm,
                            perf_mode=DR,
                        )
                        if covered:
                            nc.tensor.matmul(
                                ps[m][:, n * 512:(n + 1) * 512],
                                lhsT,
                                wlo_sb[:, cov_idx * UPG + u, :,
                                       n * 512:(n + 1) * 512],
                                start=False,
                                stop=(last_g and u == UPG - 1),
                                perf_mode=DR,
                            )
                    if last_g and u == UPG - 1:
                        pending.append((ps[m], grp * MT + m))
                        if grp == NG - 1:
                            emit_evictions()
                if g == 1 and pending:
                    emit_evictions()

            # ---- chain-unit and mm-unit streams, chain runs LOOKAHEAD ahead
            # silu(grp+1) is hoisted to mid-grp so the next group's base
            # matmuls never wait on the ACT backlog.
            chain_units = []
            for grp in range(NG):
                chain_units.append(lambda grp=grp: emit_silu(grp))
            for grp in range(NG):
                for g in range(G):
                    chain_units.append(lambda grp=grp, g=g: emit_chain(grp, g))

            mm_units = []
            for grp in range(NG):
                mm_units.append(lambda grp=grp: emit_base(grp))
                for g in range(G):
                    mm_units.append(lambda grp=grp, g=g: emit_spline(grp, g))

            # prologue: warmup junk MMs (HAM) + initial DMAs
            nc.vector.memset(ones_sb[:], 1.0)
            warm_ps = ppool.tile([128, OUT_F], F32, tag="ps", name="warm")
            for w in range(32):
                nc.tensor.matmul(
                    warm_ps[:, 0:128], ones_sb[0:1, :], ones_sb[0:1, :],
                    start=True, stop=True,
                )
            emit_xg_dma(0)
            nc.sync.dma_start(bw_sb[:], bw_d[:])
            for grp in range(1, NG):
                emit_xg_dma(grp)
            for t in range(4):
                nc.sync.dma_start(whi_sb[:, t], whi_d[:, t])
            if ncov:
                nc.sync.dma_start(wlo_sb[:], wlo_d[:])
            for t in range(4, NPAIR):
                nc.sync.dma_start(whi_sb[:, t], whi_d[:, t])

            LOOKAHEAD = 4
            ci = 0
            for j, mm in enumerate(mm_units):
                target = min(len(chain_units),
                             (j * len(chain_units)) // len(mm_units) + LOOKAHEAD)
                while ci < target:
                    chain_units[ci]()
                    ci += 1
                mm()
            while ci < len(chain_units):
                chain_units[ci]()
                ci += 1
            emit_evictions()

    nc.compile()
    return nc


def _quantize_w(spline_w):
    """e4m3 quantization of W*WSCALE with optional GPTQ-style error
    feedback across the g dimension (shared Gram, hardcoded)."""
    W = spline_w.astype(np.float64) * WSCALE  # [O, I, G]
    if not GPTQ_W:
        Whi = (W.astype(np.float32)).astype(F8NP)
        return Whi
    # Shared 8x8 Gram of the centered residual basis under x~N(0,1).
    # E[resid_g resid_g'] computed offline on the reference distribution.
    H = _RESID_GRAM + 1e-6 * np.trace(_RESID_GRAM) / G * np.eye(G)
    Hinv = np.linalg.inv(H)
    U = np.linalg.cholesky(Hinv[::-1, ::-1].T)[::-1, ::-1].T
    Wp = W.copy()
    Q = np.zeros(W.shape, dtype=F8NP)
    for k in range(G):
        Q[:, :, k] = Wp[:, :, k].astype(np.float32).astype(F8NP)
        err = Wp[:, :, k] - Q[:, :, k].astype(np.float64)
        if k + 1 < G:
            coef = U[k, k + 1:] / U[k, k]
            Wp[:, :, k + 1:] -= err[:, :, None] * coef[None, None, :]
    return Q


# E[resid_g resid_g'] for resid = basis - c - beta*silu, x ~ N(0,1).
# (filled in below by calibration; placeholder identity keeps GPTQ sane
# if calibration is skipped)
_RESID_GRAM = np.eye(G) * 0.05


def _host_prep(x, base_w, base_b, spline_w):
    x = np.asarray(x, dtype=np.float32)
    base_w = np.asarray(base_w, dtype=np.float32)
    base_b = np.asarray(base_b, dtype=np.float32)
    spline_w = np.asarray(spline_w, dtype=np.float32)

    x_flat = np.ascontiguousarray(x.reshape(TOK, IN_F)).astype(BFNP)

    # spline weights: k = g*IN + i  ->  [G*IN, OUT]
    # device resid is scaled by KDE (Derivative_Erf); divide W to compensate
    W8 = _quantize_w(spline_w / _KDE)  # [O, I, G] e4m3 (scaled by WSCALE)
    Wk_hi = W8.transpose(2, 1, 0).reshape(G * IN_F, OUT_F)

    def pack(Wm):  # [G*IN, OUT] -> [128, NPAIR, 2, OUT]
        return np.ascontiguousarray(
            Wm.reshape(NPAIR, 2, 128, OUT_F).transpose(2, 0, 1, 3))

    whi = pack(Wk_hi)

    ncov = len(LO_COVER)
    wlo = None
    if ncov:
        Wk = (spline_w / _KDE).transpose(2, 1, 0).reshape(
            G * IN_F, OUT_F).astype(np.float64)
        Wlo_full = ((Wk * WSCALE - Wk_hi.astype(np.float64))
                    .astype(np.float32).astype(F8NP))
        parts = []
        for g in sorted(LO_COVER):
            blk = Wlo_full[g * IN_F:(g + 1) * IN_F]  # [IN, OUT]
            parts.append(blk.reshape(UPG, 2, 128, OUT_F).transpose(2, 0, 1, 3))
        wlo = np.ascontiguousarray(np.concatenate(parts, axis=1))

    # base weights with beta-fold, 0.5 silu2 factor, and WSCALE
    if CENTER:
        V = np.einsum("g,oig->io", _BETA, spline_w.astype(np.float64))
    else:
        V = 0.0
    bw_eff = 0.5 * (base_w.T.astype(np.float64) + V) * WSCALE
    bw = np.ascontiguousarray(
        bw_eff.reshape(KB, 128, OUT_F).transpose(1, 0, 2)).astype(BFNP)

    if CENTER:
        bias = (_C[None, :] * spline_w.astype(np.float64).sum(axis=1)).sum(axis=1)
    else:
        bias = np.zeros(OUT_F)
    bias = bias + base_b.astype(np.float64)

    in_maps = []
    for c in range(NCORES):
        shard = x_flat[c * TCORE:(c + 1) * TCORE, :]
        xT = shard.T  # [in, tok]
        xg = np.ascontiguousarray(
            xT.reshape(KB, 128, NG, GTOK).transpose(2, 1, 0, 3))
        m = {"xg": xg, "whi": whi, "basew": bw}
        if ncov:
            m["wlo"] = wlo
        in_maps.append(m)
    return in_maps, bias


def kernel(x, base_w, base_b, spline_w):
    global _NC_CACHE, LAST_RESULT
    from concourse.bass_utils import run_bass_kernel_spmd

    in_maps, bias = _host_prep(x, base_w, base_b, spline_w)
    if _NC_CACHE is None:
        _NC_CACHE = build_nc()
    res = run_bass_kernel_spmd(
        _NC_CACHE, in_maps, core_ids=list(range(NCORES)), trace=TRACE
    )
    LAST_RESULT = res
    outs = [np.asarray(r["out"], dtype=np.float64) for r in res.results]
    full = np.concatenate(outs, axis=0) / WSCALE + bias
    return full.astype(np.float32).reshape(4, 2048, OUT_F)


# revision 23
# speedup vs baseline: 1.1071x; 1.0003x over previous
"""KANLinear (RBF-KAN) Trainium2 kernel — fp8 DoubleRow version.

Math (matches the reference):
  x_flat [B=8192, IN=1024]
  base   = silu(x) @ base_w.T + base_b
  basis[b,i,g] = exp(-(d*(x[b,i]-grid[g]))**2),  grid = linspace(-2,2,8)
  spline = einsum('big,oig->bo', basis, spline_w)
  out    = base + spline

Implementation (data-parallel over tokens, 8 cores x 1024 tokens):
  - The spline contraction runs in fp8 e4m3 with perf_mode=DoubleRow:
    each MM contracts a PAIR of 128-row k-subtiles in the time a bf16
    MM contracts one (the PE moving port is byte-bound at 2B/cycle, and
    fp8 carries 2x K per byte). Spline MMs: 32 pairs x 8 m x 2 n = 512.
  - Accuracy (plain e4m3 is ~2.7% rel err, gate 2e-2):
      1. Variance reduction: quantize the RESIDUAL basis
         resid = basis - c_g - beta_g*silu(x); (c_g, beta_g) are
         least-squares fits (hardcoded). c folds into a host bias,
         beta folds into the base weights. |resid| ~ 0.68 |basis|.
      2. W-side: error-feedback (GPTQ-style) e4m3 rounding of W across
         the 8 correlated g-rows (host-side, uses the shared 8x8 resid
         Gram), plus optional explicit W_lo correction matmuls for the
         highest-variance grid points (LO_COVER knob).
  - Per core: 4 groups x 256 tokens; PSUM holds 2 groups -> seamless
    group overlap. basis via one DVE stt (v=(x-2g)*x) + one ACT Exp;
    silu2 = x*(1+tanh(x/2)); everything stays in the exp/tanh table.
  - base_b and the c-fold bias are added on the host; outputs are
    accumulated at WSCALE and divided on the host.
"""

import os
import sys

os.environ.setdefault("MYCRO_LOCAL_CACHE", "1")
for _p in ("/opt/trn_rl_repo", "/root/.axon_site/_ro/trn_rl_repo"):
    if os.path.isdir(_p) and _p not in sys.path:
        sys.path.insert(0, _p)

import numpy as np
import ml_dtypes

F8NP = ml_dtypes.float8_e4m3
BFNP = ml_dtypes.bfloat16

IN_F = 1024
OUT_F = 1024
G = 8
GRID_LO, GRID_HI = -2.0, 2.0
NCORES = 8
TOK = 8192
TCORE = TOK // NCORES     # 1024 tokens per core
NG = 4                    # token groups per core
GTOK = TCORE // NG        # 256 tokens per group
MT = GTOK // 128          # 2 psum m-tiles per group
KB = IN_F // 128          # 8 i-blocks
KS = G * KB               # 64 k-subtiles of 128
NPAIR = KS // 2           # 32 DoubleRow k-pairs
UPG = KB // 2             # 4 pairs per g-block

WSCALE = 64.0
_KDE = 1.1283791670955126  # 2/sqrt(pi): Derivative_Erf(t) = KDE * exp(-t^2)

_DELTA = float((GRID_HI - GRID_LO) / (G - 1))
_D = 1.0 / (_DELTA + 1e-6)
_D2 = _D * _D
_GRID = np.linspace(GRID_LO, GRID_HI, G, dtype=np.float32).astype(np.float64)

# Least-squares fit of basis_g(x) ~ c_g + beta_g*silu(x) over x ~ N(0,1)
# (bf16-rounded silu). Computed offline on the reference distribution.
_C = np.array([0.08754251, 0.20408037, 0.3485522, 0.42897628,
               0.37042523, 0.21574167, 0.07760693, 0.01135657])
_BETA = np.array([-0.09874898, -0.23330925, -0.36547238, -0.32523782,
                  -0.04140068, 0.27750214, 0.37861404, 0.270346])

CENTER = True
GPTQ_W = False  # measured: no gain (resid columns ~uncorrelated across g)
LO_COVER = (3, 4)         # grid points with explicit W_lo correction MMs

TRACE = False
LAST_RESULT = None
_NC_CACHE = None


def build_nc():
    from concourse import bacc
    import concourse.mybir as mybir
    import concourse.tile as tile
    from concourse.tile_rust import add_dep_helper

    F32 = mybir.dt.float32
    BF16 = mybir.dt.bfloat16
    F8 = mybir.dt.float8e4
    Alu = mybir.AluOpType
    Act = mybir.ActivationFunctionType
    DR = mybir.MatmulPerfMode.DoubleRow

    ncov = len(LO_COVER)
    nc = bacc.Bacc("TRN2", target_bir_lowering=False)
    xg_d = nc.dram_tensor("xg", [NG, 128, KB, GTOK], BF16, kind="ExternalInput")
    whi_d = nc.dram_tensor("whi", [128, NPAIR, 2, OUT_F], F8, kind="ExternalInput")
    bw_d = nc.dram_tensor("basew", [128, KB, OUT_F], BF16, kind="ExternalInput")
    out_d = nc.dram_tensor("out", [TCORE, OUT_F], F32, kind="ExternalOutput")
    if ncov:
        wlo_d = nc.dram_tensor("wlo", [128, ncov * UPG, 2, OUT_F], F8,
                               kind="ExternalInput")

    def exp_bias(g):
        gval = float(_GRID[g])
        return float(-_D2 * gval * gval)

    # activation() requires pre-registered [128,1] const APs for fp biases
    def register_const_ap(value):
        t = nc.alloc_sbuf_tensor(f"const-bias-{value}", [128, 1], F32)
        nc.gpsimd.memset(t.ap(), value)
        nc.const_aps.aps[(F32, value)] = t.ap()

    need = {float(-_D * _GRID[g]) for g in range(G)}  # DerivErf biases
    for value in sorted(need):
        register_const_ap(value)
    nc.all_engine_barrier()

    with tile.TileContext(nc) as tc:
        with (
            tc.tile_pool(name="const", bufs=1) as cpool,
            tc.tile_pool(name="xg", bufs=4) as xpool,
            tc.tile_pool(name="silu", bufs=4) as spool,
            tc.tile_pool(name="vsc", bufs=2) as vpool,
            tc.tile_pool(name="bsc", bufs=2) as bpool,
            tc.tile_pool(name="r1s", bufs=2) as rpool,
            tc.tile_pool(name="resid", bufs=8) as fpool,
            tc.tile_pool(name="osb", bufs=1) as opool,
            tc.tile_pool(name="psum", bufs=4, space="PSUM") as ppool,
        ):
            whi_sb = cpool.tile([128, NPAIR, 2, OUT_F], F8)
            if ncov:
                wlo_sb = cpool.tile([128, ncov * UPG, 2, OUT_F], F8)
            bw_sb = cpool.tile([128, KB, OUT_F], BF16)
            ones_sb = cpool.tile([1, 128], BF16)

            pending = []

            def emit_evictions():
                for ps_t, row in pending:
                    o = opool.tile([128, OUT_F], F32, tag="osb", name=f"o_{row}")
                    nc.vector.tensor_copy(o[:, 0:512], ps_t[:, 0:512])
                    nc.scalar.copy(o[:, 512:1024], ps_t[:, 512:1024])
                    nc.sync.dma_start(out_d[row * 128:(row + 1) * 128, :], o[:])
                pending.clear()

            xg_t = {}
            silu2_t = {}
            resid_t = {}
            ps_t = {}
            tanh_insts = []
            derf_first = [True]

            def emit_xg_dma(grp, halves=1):
                xg = xpool.tile([128, KB, GTOK], BF16, tag="xg", name=f"xg{grp}")
                xg_t[grp] = xg
                if halves == 1:
                    nc.sync.dma_start(xg[:], xg_d[grp])
                else:
                    h = KB // halves
                    for i in range(halves):
                        nc.sync.dma_start(xg[:, i * h:(i + 1) * h, :],
                                          xg_d[grp, :, i * h:(i + 1) * h, :])

            def emit_silu(grp, halves=1):
                xg = xg_t[grp]
                silu2 = spool.tile([128, KB, GTOK], BF16, tag="silu",
                                   name=f"s2{grp}")
                silu2_t[grp] = silu2
                h = KB // halves
                for i in range(halves):
                    xpart = xg[:, i * h:(i + 1) * h, :].rearrange(
                        "p k t -> p (k t)")
                    th = vpool.tile([128, h * GTOK], F32, tag="vsc",
                                    name=f"th{grp}_{i}")
                    tanh_insts.append(
                        nc.scalar.activation(th[:], xpart, Act.Tanh, scale=0.5))
                    nc.vector.scalar_tensor_tensor(
                        silu2[:, i * h:(i + 1) * h, :].rearrange(
                            "p k t -> p (k t)"),
                        th[:], 1.0, xpart, op0=Alu.add, op1=Alu.mult,
                    )

            def emit_chain(grp, g):
                # basis = KDE*exp(-(d x - d g)^2) in ONE ACT op
                # (Derivative_Erf); KDE is divided out of W on the host.
                xflat = xg_t[grp][:].rearrange("p k t -> p (k t)")
                resid = fpool.tile([128, KB, GTOK], F8, tag="resid",
                                   name=f"r{grp}_{g}")
                resid_t[(grp, g)] = resid
                rflat = resid[:].rearrange("p k t -> p (k t)")
                basis = bpool.tile([128, KB * GTOK], F32, tag="bsc",
                                   name=f"b{grp}_{g}")
                derf = nc.scalar.activation(
                    basis[:], xflat, Act.Derivative_Erf,
                    bias=float(-_D * _GRID[g]), scale=float(_D),
                )
                if derf_first[0] and tanh_insts:
                    add_dep_helper(derf.ins, tanh_insts[-1].ins, sync=False,
                                   reason="batch tanh before derf (act table)")
                    derf_first[0] = False
                if CENTER:
                    # bs = KDE*((beta/2)*silu2 + c)   (Pool ts, 1-input)
                    bs = rpool.tile([128, KB * GTOK], BF16, tag="r1s",
                                    name=f"bs_{grp}_{g}")
                    nc.gpsimd.tensor_scalar(
                        bs[:], silu2_t[grp][:].rearrange("p k t -> p (k t)"),
                        float(_KDE * _BETA[g] / 2.0), float(_KDE * _C[g]),
                        op0=Alu.mult, op1=Alu.add,
                    )
                    # resid8 = basis - bs, cast e4m3 (DVE tt)
                    nc.vector.tensor_tensor(
                        rflat, basis[:], bs[:], op=Alu.subtract,
                    )
                else:
                    nc.vector.tensor_copy(rflat, basis[:])

            def emit_base(grp):
                ps = [
                    ppool.tile([128, OUT_F], F32, tag="ps",
                               name=f"ps_g{grp}m{m}")
                    for m in range(MT)
                ]
                ps_t[grp] = ps
                silu2 = silu2_t[grp]
                for m in range(MT):
                    for kb in range(KB):
                        lhsT = silu2[:, kb, m * 128:(m + 1) * 128]
                        for n in range(2):
                            nc.tensor.matmul(
                                ps[m][:, n * 512:(n + 1) * 512],
                                lhsT,
                                bw_sb[:, kb, n * 512:(n + 1) * 512],
                                start=(kb == 0), stop=False,
                            )

            def emit_spline(grp, g):
                ps = ps_t[grp]
                resid = resid_t.pop((grp, g))
                covered = g in LO_COVER
                cov_idx = sum(1 for c in LO_COVER if c < g)
                last_g = g == G - 1
                # last g-block of each group goes m-major so m0's psum can
                # evict while m1's matmuls still stream
                m_outer = last_g
                order = ([(m, u) for m in range(MT) for u in range(UPG)]
                         if m_outer else
                         [(m, u) for u in range(UPG) for m in range(MT)])
                for m, u in order:
                    t = g * UPG + u
                    lhsT = resid[:, 2 * u:2 * u + 2, m * 128:(m + 1) * 128]
                    for n in range(2):
                        last_mm = last_g and u == UPG - 1 and not covered
                        nc.tensor.matmul(
                            ps[m][:, n * 512:(n + 1) * 512],
                            lhsT,
                            whi_sb[:, t, :, n * 512:(n + 1) * 512],
                            start=False,
                            stop=last_mm,
                            perf_mode=DR,
                        )
                        if covered:
                            nc.tensor.matmul(
                                ps[m][:, n * 512:(n + 1) * 512],
                                lhsT,
                                wlo_sb[:, cov_idx * UPG + u, :,
                                       n * 512:(n + 1) * 512],
                                start=False,
                                stop=(last_g and u == UPG - 1),
                                perf_mode=DR,
                            )
                    if last_g and u == UPG - 1:
                        pending.append((ps[m], grp * MT + m))
                        if grp == NG - 1:
                            emit_evictions()
                if g == 1 and pending:
                    emit_evictions()

            # ---- chain-unit and mm-unit streams, chain runs LOOKAHEAD ahead
            # silu(grp+1) is hoisted to mid-grp so the next group's base
            # matmuls never wait on the ACT backlog.
            chain_units = []
            for grp in range(NG):
                chain_units.append(
                    lambda grp=grp: emit_silu(grp, halves=2 if grp == 0 else 1))
            for grp in range(NG):
                for g in range(G):
                    chain_units.append(lambda grp=grp, g=g: emit_chain(grp, g))

            mm_units = []
            for grp in range(NG):
                mm_units.append(lambda grp=grp: emit_base(grp))
                for g in range(G):
                    mm_units.append(lambda grp=grp, g=g: emit_spline(grp, g))

            # prologue: warmup junk MMs (HAM) + initial DMAs
            nc.vector.memset(ones_sb[:], 1.0)
            warm_ps = ppool.tile([128, OUT_F], F32, tag="ps", name="warm")
            for w in range(72):
                nc.tensor.matmul(
                    warm_ps[:, 0:128], ones_sb[0:1, :], ones_sb[0:1, :],
                    start=True, stop=True,
                )
            emit_xg_dma(0)
            nc.sync.dma_start(bw_sb[:], bw_d[:])
            for grp in range(1, NG):
                emit_xg_dma(grp)
            for t in range(4):
                nc.sync.dma_start(whi_sb[:, t], whi_d[:, t])
            if ncov:
                nc.sync.dma_start(wlo_sb[:], wlo_d[:])
            for t in range(4, NPAIR):
                nc.sync.dma_start(whi_sb[:, t], whi_d[:, t])

            LOOKAHEAD = 4
            ci = 0
            for j, mm in enumerate(mm_units):
                target = min(len(chain_units),
                             (j * len(chain_units)) // len(mm_units) + LOOKAHEAD)
                while ci < target:
                    chain_units[ci]()
                    ci += 1
                mm()
            while ci < len(chain_units):
                chain_units[ci]()
                ci += 1
            emit_evictions()

    nc.compile()
    return nc


def _quantize_w(spline_w):
    """e4m3 quantization of W*WSCALE with optional GPTQ-style error
    feedback across the g dimension (shared Gram, hardcoded)."""
    W = spline_w.astype(np.float64) * WSCALE  # [O, I, G]
    if not GPTQ_W:
        Whi = (W.astype(np.float32)).astype(F8NP)
        return Whi
    # Shared 8x8 Gram of the centered residual basis under x~N(0,1).
    # E[resid_g resid_g'] computed offline on the reference distribution.
    H = _RESID_GRAM + 1e-6 * np.trace(_RESID_GRAM) / G * np.eye(G)
    Hinv = np.linalg.inv(H)
    U = np.linalg.cholesky(Hinv[::-1, ::-1].T)[::-1, ::-1].T
    Wp = W.copy()
    Q = np.zeros(W.shape, dtype=F8NP)
    for k in range(G):
        Q[:, :, k] = Wp[:, :, k].astype(np.float32).astype(F8NP)
        err = Wp[:, :, k] - Q[:, :, k].astype(np.float64)
        if k + 1 < G:
            coef = U[k, k + 1:] / U[k, k]
            Wp[:, :, k + 1:] -= err[:, :, None] * coef[None, None, :]
    return Q


# E[resid_g resid_g'] for resid = basis - c - beta*silu, x ~ N(0,1).
# (filled in below by calibration; placeholder identity keeps GPTQ sane
# if calibration is skipped)
_RESID_GRAM = np.eye(G) * 0.05


def _host_prep(x, base_w, base_b, spline_w):
    x = np.asarray(x, dtype=np.float32)
    base_w = np.asarray(base_w, dtype=np.float32)
    base_b = np.asarray(base_b, dtype=np.float32)
    spline_w = np.asarray(spline_w, dtype=np.float32)

    x_flat = np.ascontiguousarray(x.reshape(TOK, IN_F)).astype(BFNP)

    # spline weights: k = g*IN + i  ->  [G*IN, OUT]
    # device resid is scaled by KDE (Derivative_Erf); divide W to compensate
    W8 = _quantize_w(spline_w / _KDE)  # [O, I, G] e4m3 (scaled by WSCALE)
    Wk_hi = W8.transpose(2, 1, 0).reshape(G * IN_F, OUT_F)

    def pack(Wm):  # [G*IN, OUT] -> [128, NPAIR, 2, OUT]
        return np.ascontiguousarray(
            Wm.reshape(NPAIR, 2, 128, OUT_F).transpose(2, 0, 1, 3))

    whi = pack(Wk_hi)

    ncov = len(LO_COVER)
    wlo = None
    if ncov:
        Wk = (spline_w / _KDE).transpose(2, 1, 0).reshape(
            G * IN_F, OUT_F).astype(np.float64)
        Wlo_full = ((Wk * WSCALE - Wk_hi.astype(np.float64))
                    .astype(np.float32).astype(F8NP))
        parts = []
        for g in sorted(LO_COVER):
            blk = Wlo_full[g * IN_F:(g + 1) * IN_F]  # [IN, OUT]
            parts.append(blk.reshape(UPG, 2, 128, OUT_F).transpose(2, 0, 1, 3))
        wlo = np.ascontiguousarray(np.concatenate(parts, axis=1))

    # base weights with beta-fold, 0.5 silu2 factor, and WSCALE
    if CENTER:
        V = np.einsum("g,oig->io", _BETA, spline_w.astype(np.float64))
    else:
        V = 0.0
    bw_eff = 0.5 * (base_w.T.astype(np.float64) + V) * WSCALE
    bw = np.ascontiguousarray(
        bw_eff.reshape(KB, 128, OUT_F).transpose(1, 0, 2)).astype(BFNP)

    if CENTER:
        bias = (_C[None, :] * spline_w.astype(np.float64).sum(axis=1)).sum(axis=1)
    else:
        bias = np.zeros(OUT_F)
    bias = bias + base_b.astype(np.float64)

    in_maps = []
    for c in range(NCORES):
        shard = x_flat[c * TCORE:(c + 1) * TCORE, :]
        xT = shard.T  # [in, tok]
        xg = np.ascontiguousarray(
            xT.reshape(KB, 128, NG, GTOK).transpose(2, 1, 0, 3))
        m = {"xg": xg, "whi": whi, "basew": bw}
        if ncov:
            m["wlo"] = wlo
        in_maps.append(m)
    return in_maps, bias


def kernel(x, base_w, base_b, spline_w):
    global _NC_CACHE, LAST_RESULT
    from concourse.bass_utils import run_bass_kernel_spmd

    in_maps, bias = _host_prep(x, base_w, base_b, spline_w)
    if _NC_CACHE is None:
        _NC_CACHE = build_nc()
    res = run_bass_kernel_spmd(
        _NC_CACHE, in_maps, core_ids=list(range(NCORES)), trace=TRACE
    )
    LAST_RESULT = res
    outs = [np.asarray(r["out"], dtype=np.float64) for r in res.results]
    full = np.concatenate(outs, axis=0) / WSCALE + bias
    return full.astype(np.float32).reshape(4, 2048, OUT_F)
